# revision 2
# baseline (speedup 1.0000x reference)
"""Multi-head causal self-attention (B=2, T=2048, D=1024, H=16, Dh=64) on 8
Trainium2 NeuronCores.

Sharding (Megatron-style tensor parallel over heads):
  - Each core owns 2 heads (core c -> heads 2c, 2c+1) for both batch rows.
  - w_qkv column-sharded per core ([1024, 128] per q/k/v, bf16 on host).
  - w_proj row-sharded ([128, 1024] bf16); cores emit partial projection
    outputs which the host sums (plus bias terms folded exactly on host).
  - x replicated, passed pre-transposed AND pre-cast: xT [1024, 4096] bf16
    (halves the input DMA vs fp32 and removes the on-device cast).

Device-side per core:
  xT -> qT/kT/vT = W^T x^T via PE (fp32 PSUM).
  q evicted as fp8e4 (q8); k evicted as fp8e4 pair (k8, k8r) where
  k8r = (k + bk) - k8 is the quantization residual, stored slot-interleaved
  in one tile. Scores use fp8 DoubleRow matmuls (0.5 cyc/col): the two
  stationary tiles are (k8, k8r) and the moving tiles are (q8, q8)
  via a stride-0 broadcast AP, computing (k8+k8r)^T q8 -- K is accurate to
  ~13 bits, Q to e4m3, so softmax logits carry ~2.5% relative error on a
  0.41-std logit, well inside the 2e-2 budget.
  The causal mask for the two diagonal chunks is folded in as fp8e5
  identity.T @ (-1536 triangle) DoubleRow accumulates; the odd-diagonal
  chunk only computes its valid 128-query half, and PV skips that chunk
  for the first query sub-block.
  exp on ACT per 2-chunk PSUM bank -> P tiles (bf16); PV in bf16 with a
  prepended ones-column in V2 yielding softmax denominators; normalize on
  DVE; 4 PE transposes packed per PSUM bank (tile_position places head 1
  at partitions 64..127) with a single DVE evict into the projection
  layout; y_partial = attn_out @ w_proj_slice.

Schedule: qkv T-chunks are interleaved through BOTH batches' attention
(tc j before blocks 2j, 2j+1) because late attention blocks are
ACT(exp)-bound while qkv is pure PE work; block outputs are software-
pipelined at distance 2 behind their scores.
"""

import numpy as np
import ml_dtypes

import concourse.bacc as bacc
import concourse.bass as bass
import concourse.mybir as mybir
import concourse.tile as tile
from concourse.bass_utils import run_bass_kernel_spmd
from concourse.masks import make_identity

N_CORES = 8
B = 2
T = 2048
D = 1024
H = 16
DH = 64
TA = B * T  # 4096 rows total
P = 128
NQB = T // P  # 16 key chunks per batch
KC = D // P  # 8 contraction chunks for qkv
SQ = 256  # superblock query count
BF = mybir.dt.bfloat16
F32 = mybir.dt.float32
F8 = mybir.dt.float8e4
F8E5 = mybir.dt.float8e5
DR = mybir.MatmulPerfMode.DoubleRow

_CACHED_NC = None


def build_nc():
    """Build the per-core Bass program (identical on all 8 cores)."""
    nc = bacc.Bacc("TRN2", target_bir_lowering=False, debug=False, num_devices=N_CORES)

    xT_in = nc.dram_tensor("xT", [D, TA], BF, kind="ExternalInput").ap()
    wq_in = nc.dram_tensor("wq", [D, P], BF, kind="ExternalInput").ap()
    wk_in = nc.dram_tensor("wk", [D, P], BF, kind="ExternalInput").ap()
    wv_in = nc.dram_tensor("wv", [D, P], BF, kind="ExternalInput").ap()
    bq_in = nc.dram_tensor("bq", [P, 1], F32, kind="ExternalInput").ap()
    bk_in = nc.dram_tensor("bk", [P, 1], F32, kind="ExternalInput").ap()
    wp_in = nc.dram_tensor("wp", [P, D], BF, kind="ExternalInput").ap()
    i8_in = nc.dram_tensor("i8", [P, 2, P], F8E5, kind="ExternalInput").ap()
    m8_in = nc.dram_tensor("m8", [P, 2, P], F8E5, kind="ExternalInput").ap()
    y_out = nc.dram_tensor("y", [TA, D], BF, kind="ExternalOutput").ap()

    with tile.TileContext(nc) as tc:
        with (
            tc.tile_pool(name="const", bufs=1) as const,
            tc.tile_pool(name="xts", bufs=1) as xts,
            tc.tile_pool(name="qkv", bufs=1) as qkv,
            tc.tile_pool(name="ptp", bufs=6) as ptp,
            tc.tile_pool(name="osml", bufs=8) as osml,
            tc.tile_pool(name="rcp", bufs=8) as rcp,
            tc.tile_pool(name="ystage", bufs=4) as ystage,
            tc.tile_pool(name="ps_mm", bufs=2, space="PSUM") as ps_mm,
            tc.tile_pool(name="ps_st", bufs=3, space="PSUM") as ps_st,
            tc.tile_pool(name="ps_sm", bufs=3, space="PSUM") as ps_sm,
        ):
            # ---- constants (wp/mask DMAs deferred below x split 0) ----
            ident = const.tile([P, P], BF)
            make_identity(nc, ident[:])
            bq_sb = const.tile([P, 1], F32)
            nc.sync.dma_start(bq_sb[:], bq_in[:])
            bk_sb = const.tile([P, 1], F32)
            nc.sync.dma_start(bk_sb[:], bk_in[:])
            # qkv weight chunks as matmul lhsT tiles [K=128 D-rows, 128 feats]
            w_sb = {}
            for name, ap in (("q", wq_in), ("k", wk_in), ("v", wv_in)):
                w = const.tile([P, KC, P], BF, name=f"w{name}sb")
                for c in range(KC):
                    nc.sync.dma_start(w[:, c, :], ap[c * P : (c + 1) * P, :])
                w_sb[name] = w

            # ---- xT load (already bf16 from host) ----
            xT_sb = xts.tile([P, KC, TA], BF)
            NSPLIT = 8
            SW = TA // NSPLIT  # 512 cols per split
            i8_sb = const.tile([P, 2, P], F8E5)
            m8_sb = const.tile([P, 2, P], F8E5)
            wp_sb = const.tile([P, D], BF)
            for s in range(NSPLIT):
                for c in range(KC):
                    dma_eng = nc.scalar if s == 0 else nc.sync
                    dma_eng.dma_start(
                        xT_sb[:, c, s * SW : (s + 1) * SW],
                        xT_in[c * P : (c + 1) * P, s * SW : (s + 1) * SW],
                    )
                if s == 0:  # needed later than qkv; keep off the startup queue
                    nc.sync.dma_start(i8_sb[:], i8_in[:])
                    nc.sync.dma_start(m8_sb[:], m8_in[:])
                    nc.sync.dma_start(wp_sb[:], wp_in[:])

            # ---- PE warmup: dependency-free matmuls keep the array busy
            # through the DMA-gated x-load ramp so HAM reaches 2.4 GHz
            # before the first real qkv matmul ----
            wm = ps_mm.tile([P, 512], F32, name="warm", tag="psq")
            for _ in range(72):
                nc.tensor.matmul(
                    wm[:, 0:P], ident[:], ident[:], start=True, stop=True
                )

            # ---- qkv projections ----
            # q8_sb: fp8(q + bq); k2_sb slot 0 = fp8(k + bk), slot 1 = residual
            q8_sb = qkv.tile([P, B, T], F8)
            k2_sb = qkv.tile([P, 2, B, T], F8)
            vT_sb = qkv.tile([P, B, T], BF)
            # V2 per (b, key-chunk): [1 | V_h0 (64) | 1 | V_h1 (64)]
            V2 = qkv.tile([P, B, NQB, 130], BF)
            nc.vector.memset(V2[:, :, :, 0], 1.0)
            nc.vector.memset(V2[:, :, :, 65], 1.0)
            attn_oT = qkv.tile([P, TA], BF)
            NTC = TA // 512  # 8 T-chunks of 512

            def qkv_tchunk(tcg):
                b = tcg // (NTC // B)
                col = (tcg % (NTC // B)) * 512
                # q -> fp8 with bias
                pq = ps_mm.tile([P, 512], F32, name="pq", tag="psq")
                for c in range(KC):
                    nc.tensor.matmul(
                        pq[:],
                        w_sb["q"][:, c, :],
                        xT_sb[:, c, tcg * 512 : (tcg + 1) * 512],
                        start=(c == 0),
                        stop=(c == KC - 1),
                    )
                nc.vector.tensor_scalar(
                    q8_sb[:, b, col : col + 512],
                    pq[:],
                    bq_sb[:],
                    None,
                    op0=mybir.AluOpType.add,
                )
                # k -> fp8 + residual
                pk = ps_mm.tile([P, 512], F32, name="pk", tag="psq")
                for c in range(KC):
                    nc.tensor.matmul(
                        pk[:],
                        w_sb["k"][:, c, :],
                        xT_sb[:, c, tcg * 512 : (tcg + 1) * 512],
                        start=(c == 0),
                        stop=(c == KC - 1),
                    )
                k8 = k2_sb[:, 0, b, col : col + 512]
                nc.vector.tensor_scalar(
                    k8, pk[:], bk_sb[:], None, op0=mybir.AluOpType.add
                )
                nc.vector.scalar_tensor_tensor(
                    k2_sb[:, 1, b, col : col + 512],
                    pk[:],
                    bk_sb[:],
                    k8,
                    op0=mybir.AluOpType.add,
                    op1=mybir.AluOpType.subtract,
                )
                # v -> bf16
                pv_ = ps_mm.tile([P, 512], F32, name="pvq", tag="psq")
                for c in range(KC):
                    nc.tensor.matmul(
                        pv_[:],
                        w_sb["v"][:, c, :],
                        xT_sb[:, c, tcg * 512 : (tcg + 1) * 512],
                        start=(c == 0),
                        stop=(c == KC - 1),
                    )
                nc.vector.tensor_copy(vT_sb[:, b, col : col + 512], pv_[:])
                # V fixup: 4 transposes packed into one PSUM bank + 1 evict
                bs = (tcg % (NTC // B)) * 4
                vtp = ps_mm.tile([P, 4, P], BF, name="vtp", tag="psq")
                for j in range(4):
                    s = bs + j
                    nc.tensor.matmul(
                        vtp[:, j, :],
                        vT_sb[:, b, s * P : (s + 1) * P],
                        ident[:],
                        is_transpose=True,
                        start=(j == 0),
                        stop=(j == 3),
                    )
                nc.vector.tensor_copy(
                    V2[:, b, bs : bs + 4, :].rearrange(
                        "p s (h x) -> p s h x", h=2
                    )[:, :, :, 1:65],
                    vtp[:].rearrange("p s (h d) -> p s h d", h=2),
                )

            def proj_tchunk(tt):
                # y_partial rows [128*tt, 128*tt+128) = attn_out @ w_proj_slice
                ys = ystage.tile([P, D], BF)
                for nh in range(2):
                    psp = ps_mm.tile([P, 512], F32, name="psp", tag="psq")
                    nc.tensor.matmul(
                        psp[:],
                        attn_oT[:, tt * P : (tt + 1) * P],
                        wp_sb[:, nh * 512 : (nh + 1) * 512],
                        start=True,
                        stop=True,
                    )
                    nc.vector.tensor_copy(ys[:, nh * 512 : (nh + 1) * 512], psp[:])
                nc.sync.dma_start(y_out[tt * P : (tt + 1) * P, :], ys[:])

            def attn_scores(b, sq):
                """S^T fp8 DoubleRow matmuls + exp for one 256-query superblock.

                Chunks [128 keys, 256 queries]; lhsT tiles = (k8, k8r), moving
                tiles = (q8, q8) via stride-0 broadcast. Odd-diagonal chunk
                computes only its valid second 128-query half; causal masks
                fold in as fp8e5 ident.T @ (-1536 tri) DoubleRow accumulates.
                """
                nk = 2 * sq + 2
                q0 = sq * SQ
                pt = {}
                for h in (0, 1):
                    pt[h] = ptp.tile([P, NQB, SQ], BF, name="ptt", tag="pt")
                for g in range(0, nk, 2):  # PSUM groups of 2 chunks
                    diag = g == nk - 2
                    st = {}
                    for h in (0, 1):
                        st[h] = ps_st.tile([P, 512], F32, name="st", tag="st")
                    for j in (0, 1):
                        c = g + j
                        odd_diag = diag and j == 1
                        for h in (0, 1):
                            hp = h * DH
                            lhsT = k2_sb[hp : hp + DH, :, b, c * P : (c + 1) * P]
                            if odd_diag:
                                q8b = (
                                    q8_sb[hp : hp + DH, b, q0 + P : q0 + SQ]
                                    .unsqueeze(1)
                                    .broadcast_to([DH, 2, P])
                                )
                                dst = st[h][:, j * SQ + P : (j + 1) * SQ]
                            else:
                                q8b = (
                                    q8_sb[hp : hp + DH, b, q0 : q0 + SQ]
                                    .unsqueeze(1)
                                    .broadcast_to([DH, 2, SQ])
                                )
                                dst = st[h][:, j * SQ : (j + 1) * SQ]
                            nc.tensor.matmul(
                                dst,
                                lhsT,
                                q8b,
                                start=(j == 0),
                                stop=(j == 1) and not diag,
                                perf_mode=DR,
                            )
                    if diag:
                        # even-diag: triangular mask on first 128 queries;
                        # odd-diag: triangular mask on its 128 valid queries
                        for h in (0, 1):
                            nc.tensor.matmul(
                                st[h][:, 0:P],
                                i8_sb[:],
                                m8_sb[:],
                                start=False,
                                stop=False,
                                perf_mode=DR,
                            )
                            nc.tensor.matmul(
                                st[h][:, SQ + P : 2 * SQ],
                                i8_sb[:],
                                m8_sb[:],
                                start=False,
                                stop=True,
                                perf_mode=DR,
                            )
                    for h in (0, 1):
                        nc.scalar.activation(
                            pt[h][:, g : g + 2, :],
                            st[h][:, 0 : 2 * SQ],
                            mybir.ActivationFunctionType.Exp,
                            scale=0.125,
                        )
                return pt

            def attn_output(b, sq, pt):
                """PV + normalize + packed PE transposes per superblock."""
                nk = 2 * sq + 2
                # PE stage 1: 4 PV chains back-to-back into one PSUM bank
                pvt = ps_sm.tile([P, 4 * 65], F32, name="pvt", tag="sm")
                first = True
                for h in (0, 1):
                    for qh in (0, 1):
                        off = (2 * h + qh) * 65
                        nck = nk - 1 if qh == 0 else nk  # skip masked odd-diag
                        for c in range(nck):
                            nc.tensor.matmul(
                                pvt[:, off : off + 65],
                                pt[h][:, c, qh * P : (qh + 1) * P],
                                V2[:, b, c, h * 65 : h * 65 + 65],
                                start=first,
                                stop=(c == nck - 1),
                            )
                            first = False
                # DVE stage: normalize (denominator in col 0 of each chain)
                osbs = []
                for h in (0, 1):
                    for qh in (0, 1):
                        off = (2 * h + qh) * 65
                        r = rcp.tile([P, 1], F32, name="rr", tag="rr")
                        nc.vector.reciprocal(r[:], pvt[:, off : off + 1])
                        osb = osml.tile([P, DH], BF)
                        nc.vector.tensor_scalar_mul(
                            osb[:], pvt[:, off + 1 : off + 65], r[:]
                        )
                        osbs.append((h, qh, osb))
                # PE stage 2: 4 transposes packed into one bank + 1 DVE evict
                tp = ps_sm.tile([P, SQ], BF, name="tp", tag="sm")
                for h, qh, osb in osbs:
                    hp = h * DH
                    nc.tensor.matmul(
                        tp[hp : hp + DH, qh * P : (qh + 1) * P],
                        osb[:],
                        ident[:],
                        is_transpose=True,
                        tile_position=(0, hp),
                        start=(qh == 0),
                        stop=(qh == 1),
                    )
                nc.vector.tensor_copy(
                    attn_oT[:, b * T + sq * SQ : b * T + (sq + 1) * SQ], tp[:]
                )

            # ---- emission: tc j feeds blocks (b, 2j), (b, 2j+1); outputs
            # run 2 blocks behind their scores so ACT exp of block i overlaps
            # PE PV/proj of block i-2 ----
            pending = []

            def drain_one():
                pb, psq, ppt = pending.pop(0)
                attn_output(pb, psq, ppt)
                proj_tchunk(pb * (TA // P // B) + 2 * psq)
                proj_tchunk(pb * (TA // P // B) + 2 * psq + 1)

            for b in range(B):
                for j in range(NTC // B):
                    qkv_tchunk(b * (NTC // B) + j)
                    for sq in (2 * j, 2 * j + 1):
                        pt = attn_scores(b, sq)
                        pending.append((b, sq, pt))
                        if len(pending) > 2:
                            drain_one()
            while pending:
                drain_one()

    nc.compile()
    return nc


def get_nc():
    global _CACHED_NC
    if _CACHED_NC is None:
        _CACHED_NC = build_nc()
    return _CACHED_NC


def make_in_maps(x, w_qkv, b_qkv, w_proj):
    bf = ml_dtypes.bfloat16
    e5 = ml_dtypes.float8_e5m2
    x = np.asarray(x, dtype=np.float32).reshape(TA, D)
    w_qkv = np.asarray(w_qkv, dtype=np.float32)
    b_qkv = np.asarray(b_qkv, dtype=np.float32)
    w_proj = np.asarray(w_proj, dtype=np.float32)
    xT = np.ascontiguousarray(x.T).astype(bf)  # [D, TA] bf16, replicated
    # fp8e5 identity + strict-upper-triangular additive mask (-1536): the
    # same [128,128] triangle serves both diagonal chunks. Slot 1 of each
    # DoubleRow pair is zeros.
    i8 = np.zeros((P, 2, P), dtype=e5)
    i8[:, 0, :] = np.eye(P, dtype=np.float32).astype(e5)
    m8 = np.zeros((P, 2, P), dtype=e5)
    kk = np.arange(P)[:, None]
    qq = np.arange(P)[None, :]
    m8[:, 0, :] = np.where(kk > qq, -1536.0, 0.0).astype(e5)
    in_maps = []
    for c in range(N_CORES):
        lo = 2 * c * DH  # first feature column of this core's 2 heads
        in_maps.append(
            {
                "xT": xT,
                "wq": np.ascontiguousarray(w_qkv[:, lo : lo + P]).astype(bf),
                "wk": np.ascontiguousarray(w_qkv[:, D + lo : D + lo + P]).astype(bf),
                "wv": np.ascontiguousarray(
                    w_qkv[:, 2 * D + lo : 2 * D + lo + P]
                ).astype(bf),
                "bq": np.ascontiguousarray(b_qkv[lo : lo + P][:, None]),
                "bk": np.ascontiguousarray(b_qkv[D + lo : D + lo + P][:, None]),
                "wp": np.ascontiguousarray(w_proj[lo : lo + P, :]).astype(bf),
                "i8": i8,
                "m8": m8,
            }
        )
    return in_maps


def gather(results, b_qkv, w_proj, b_proj):
    b_qkv = np.asarray(b_qkv, dtype=np.float32)
    w_proj = np.asarray(w_proj, dtype=np.float32)
    b_proj = np.asarray(b_proj, dtype=np.float32)
    y = np.zeros((TA, D), dtype=np.float32)
    for c in range(N_CORES):
        y += np.asarray(results[c]["y"], dtype=np.float32)
    # exact host-side fold of the v-bias and projection bias:
    # softmax rows sum to 1, so the v-bias passes through attention intact.
    y += b_qkv[2 * D : 3 * D] @ w_proj + b_proj
    return y.reshape(B, T, D)


def run(x, w_qkv, b_qkv, w_proj, b_proj, trace=False, **spmd_kwargs):
    nc = get_nc()
    in_maps = make_in_maps(x, w_qkv, b_qkv, w_proj)
    res = run_bass_kernel_spmd(
        nc, in_maps, list(range(N_CORES)), trace=trace, **spmd_kwargs
    )
    return gather(res.results, b_qkv, w_proj, b_proj), res


def kernel(x, w_qkv, b_qkv, w_proj, b_proj):
    y, _ = run(x, w_qkv, b_qkv, w_proj, b_proj)
    return y


# revision 3
# speedup vs baseline: 1.1722x; 1.1722x over previous
"""Multi-head causal self-attention (B=2, T=2048, D=1024, H=16, Dh=64) on 8
Trainium2 NeuronCores.

Sharding (Megatron-style tensor parallel over heads):
  - Each core owns 2 heads (core c -> heads 2c, 2c+1) for both batch rows.
  - w_qkv column-sharded per core ([1024, 128] per q/k/v, bf16 on host).
  - w_proj row-sharded ([128, 1024] bf16); cores emit partial projection
    outputs which the host sums (plus bias terms folded exactly on host).
  - x replicated, passed pre-transposed AND pre-cast: xT [1024, 4096] bf16
    (halves the input DMA vs fp32 and removes the on-device cast).

Device-side per core:
  qT/kT/vT = W^T x^T on PE. q evicted as fp8e4 (q8); k as an fp8e4 pair
  (k8, k8r) slot-interleaved in one tile, k8r = (k + bk) - k8 being the
  quantization residual. Scores are fp8 DoubleRow matmuls (0.5 cyc/col):
  stationary tiles (k8, k8r), moving tiles (q8, q8) via a stride-0
  broadcast AP -> (k8+k8r)^T q8: K accurate to ~13 bits, Q to e4m3, logit
  error ~2.5% of a 0.41-std logit => ~1.2e-2 final rel err, inside 2e-2.
  Causal masks for the two diagonal chunks fold in as fp8e5
  identity.T @ (-1536 triangle) DoubleRow accumulates; the odd-diagonal
  chunk computes only its valid 128-query half and PV skips it for the
  first query sub-block. exp on ACT per 2-chunk PSUM bank; PV in bf16
  with a ones-column in V2 producing softmax denominators in col 0;
  normalize on DVE; 4 PE transposes packed per PSUM bank (tile_position
  places head 1 at partitions 64..127) + one DVE evict; projection per
  256-row block with a single batched y DMA.

Schedule: ACT (exp) is the second-busiest engine (~88us) after PE
(~93us) and exp backlog is capped by 3 score PSUM banks (~1.8us), so any
contiguous >2us stretch of non-score PE work starves ACT. The emitter
therefore weaves: score groups are emitted back-to-back while qkv chains
(split into ~0.9us pieces) and block output work (PV/normalize/proj,
pipelined >=2 blocks behind) are popped from a filler queue between
groups at a rate proportional to the remaining filler/group ratio.
Forced drains keep feasibility: tc j before the blocks that read it, and
output units before their pt pool slots are reused (4-block window).
"""

import numpy as np
import ml_dtypes

import concourse.bacc as bacc
import concourse.bass as bass
import concourse.mybir as mybir
import concourse.tile as tile
from concourse.bass_utils import run_bass_kernel_spmd
from concourse.masks import make_identity

N_CORES = 8
B = 2
T = 2048
D = 1024
H = 16
DH = 64
TA = B * T  # 4096 rows total
P = 128
NQB = T // P  # 16 key chunks per batch
KC = D // P  # 8 contraction chunks for qkv
SQ = 256  # superblock query count
BF = mybir.dt.bfloat16
F32 = mybir.dt.float32
F8 = mybir.dt.float8e4
F8E5 = mybir.dt.float8e5
DR = mybir.MatmulPerfMode.DoubleRow
ADD = mybir.AluOpType.add
SUB = mybir.AluOpType.subtract

_CACHED_NC = None


def build_nc():
    """Build the per-core Bass program (identical on all 8 cores)."""
    nc = bacc.Bacc("TRN2", target_bir_lowering=False, debug=False, num_devices=N_CORES)

    xT_in = nc.dram_tensor("xT", [D, TA], BF, kind="ExternalInput").ap()
    wq_in = nc.dram_tensor("wq", [D, P], BF, kind="ExternalInput").ap()
    wk_in = nc.dram_tensor("wk", [D, P], BF, kind="ExternalInput").ap()
    wv_in = nc.dram_tensor("wv", [D, P], BF, kind="ExternalInput").ap()
    bq_in = nc.dram_tensor("bq", [P, 1], F32, kind="ExternalInput").ap()
    bk_in = nc.dram_tensor("bk", [P, 1], F32, kind="ExternalInput").ap()
    wp_in = nc.dram_tensor("wp", [P, D], BF, kind="ExternalInput").ap()
    i8_in = nc.dram_tensor("i8", [P, 2, P], F8E5, kind="ExternalInput").ap()
    m8_in = nc.dram_tensor("m8", [P, 2, P], F8E5, kind="ExternalInput").ap()
    y_out = nc.dram_tensor("y", [TA, D], BF, kind="ExternalOutput").ap()

    with tile.TileContext(nc) as tc:
        with (
            tc.tile_pool(name="const", bufs=1) as const,
            tc.tile_pool(name="xts", bufs=1) as xts,
            tc.tile_pool(name="qkv", bufs=1) as qkv,
            tc.tile_pool(name="ptp", bufs=8) as ptp,
            tc.tile_pool(name="osml", bufs=8) as osml,
            tc.tile_pool(name="rcp", bufs=8) as rcp,
            tc.tile_pool(name="ystage", bufs=3) as ystage,
            tc.tile_pool(name="ps_st", bufs=3, space="PSUM") as ps_st,
            tc.tile_pool(name="ps_qk", bufs=1, space="PSUM") as ps_qk,
            tc.tile_pool(name="ps_pj", bufs=2, space="PSUM") as ps_pj,
            tc.tile_pool(name="ps_sm", bufs=2, space="PSUM") as ps_sm,
        ):
            # ---- constants ----
            ident = const.tile([P, P], BF)
            make_identity(nc, ident[:])
            bq_sb = const.tile([P, 1], F32)
            nc.sync.dma_start(bq_sb[:], bq_in[:])
            bk_sb = const.tile([P, 1], F32)
            nc.sync.dma_start(bk_sb[:], bk_in[:])
            # qkv weight chunks as matmul lhsT tiles, one DMA per weight
            w_sb = {}
            for name, ap in (("q", wq_in), ("k", wk_in), ("v", wv_in)):
                w = const.tile([P, KC, P], BF, name=f"w{name}sb")
                nc.sync.dma_start(w[:], ap.rearrange("(c p) f -> p c f", c=KC))
                w_sb[name] = w

            # ---- xT load (bf16 from host), 1024-col splits ----
            xT_sb = xts.tile([P, KC, TA], BF)
            NSPLIT = 4
            SW = TA // NSPLIT
            i8_sb = const.tile([P, 2, P], F8E5)
            m8_sb = const.tile([P, 2, P], F8E5)
            wp_sb = const.tile([P, D], BF)
            for s in range(NSPLIT):
                for c in range(KC):
                    dma_eng = nc.scalar if s == 0 else nc.sync
                    dma_eng.dma_start(
                        xT_sb[:, c, s * SW : (s + 1) * SW],
                        xT_in[c * P : (c + 1) * P, s * SW : (s + 1) * SW],
                    )
                if s == 0:  # needed later than qkv; keep off the startup queue
                    nc.sync.dma_start(i8_sb[:], i8_in[:])
                    nc.sync.dma_start(m8_sb[:], m8_in[:])
                    nc.sync.dma_start(wp_sb[:], wp_in[:])

            # ---- PE warmup: dependency-free matmuls cover the x-load ramp
            # so HAM reaches 2.4 GHz before the first real qkv matmul ----
            wm = ps_pj.tile([P, 512], F32, name="warm", tag="pj")
            for _ in range(80):
                nc.tensor.matmul(
                    wm[:, 0:P], ident[:], ident[:], start=True, stop=True
                )

            # ---- persistent activation tiles ----
            q8_sb = qkv.tile([P, B, T], F8)
            k2_sb = qkv.tile([P, 2, B, T], F8)
            vT_sb = qkv.tile([P, B, T], BF)
            # V2 per (b, key-chunk): [1 | V_h0 (64) | 1 | V_h1 (64)]
            V2 = qkv.tile([P, B, NQB, 130], BF)
            nc.vector.memset(V2[:, :, :, 0], 1.0)
            nc.vector.memset(V2[:, :, :, 65], 1.0)
            attn_oT = qkv.tile([P, TA], BF)

            # ---- qkv T-chunk as 7 filler pieces (~0.9us PE each) ----
            def make_tc_pieces(tcg):
                b = tcg // 4
                col = (tcg % 4) * 512
                held = {}

                def chain_a(blk):
                    def f():
                        pst = ps_qk.tile([P, 512], F32, name="pqk", tag="qk")
                        held[blk] = pst
                        for c in range(4):
                            nc.tensor.matmul(
                                pst[:],
                                w_sb[blk][:, c, :],
                                xT_sb[:, c, tcg * 512 : (tcg + 1) * 512],
                                start=(c == 0),
                                stop=False,
                            )

                    return f

                def chain_b(blk):
                    def f():
                        pst = held[blk]
                        for c in range(4, KC):
                            nc.tensor.matmul(
                                pst[:],
                                w_sb[blk][:, c, :],
                                xT_sb[:, c, tcg * 512 : (tcg + 1) * 512],
                                start=False,
                                stop=(c == KC - 1),
                            )
                        if blk == "q":
                            nc.vector.tensor_scalar(
                                q8_sb[:, b, col : col + 512],
                                pst[:],
                                bq_sb[:],
                                None,
                                op0=ADD,
                            )
                        elif blk == "k":
                            k8 = k2_sb[:, 0, b, col : col + 512]
                            nc.vector.tensor_scalar(
                                k8, pst[:], bk_sb[:], None, op0=ADD
                            )
                            nc.vector.scalar_tensor_tensor(
                                k2_sb[:, 1, b, col : col + 512],
                                pst[:],
                                bk_sb[:],
                                k8,
                                op0=ADD,
                                op1=SUB,
                            )
                        else:
                            nc.vector.tensor_copy(
                                vT_sb[:, b, col : col + 512], pst[:]
                            )

                    return f

                def vfix():
                    bs = (tcg % 4) * 4
                    vtp = ps_qk.tile([P, 4, P], BF, name="vtp", tag="qk")
                    for j in range(4):
                        s = bs + j
                        nc.tensor.matmul(
                            vtp[:, j, :],
                            vT_sb[:, b, s * P : (s + 1) * P],
                            ident[:],
                            is_transpose=True,
                            start=(j == 0),
                            stop=(j == 3),
                        )
                    nc.vector.tensor_copy(
                        V2[:, b, bs : bs + 4, :].rearrange(
                            "p s (h x) -> p s h x", h=2
                        )[:, :, :, 1:65],
                        vtp[:].rearrange("p s (h d) -> p s h d", h=2),
                    )

                return [
                    chain_a("q"),
                    chain_b("q"),
                    chain_a("k"),
                    chain_b("k"),
                    chain_a("v"),
                    chain_b("v"),
                    vfix,
                ]

            # ---- filler queue: (deadline_block_idx, closure) FIFO ----
            filler = []
            groups_total = 2 * sum(sq + 1 for sq in range(8))
            groups_done = [0]

            def pop_fillers(k):
                for _ in range(min(k, len(filler))):
                    filler.pop(0)[1]()

            def force_drain(upto):
                while filler and filler[0][0] <= upto:
                    filler.pop(0)[1]()

            def weave():
                groups_done[0] += 1
                left = groups_total - groups_done[0]
                if left <= 0 or not filler:
                    return
                pop_fillers(min(-(-len(filler) // left), 5))

            # ---- scores for one 256-query superblock (fp8 DoubleRow) ----
            def emit_scores(b, sq):
                nk = 2 * sq + 2
                q0 = b * T + sq * SQ
                pt = {}
                for h in (0, 1):
                    pt[h] = ptp.tile([P, NQB, SQ], BF, name="ptt", tag="pt")
                for g in range(0, nk, 2):
                    diag = g == nk - 2
                    st = {}
                    for h in (0, 1):
                        st[h] = ps_st.tile([P, 512], F32, name="st", tag="st")
                    for j in (0, 1):
                        c = g + j
                        odd_diag = diag and j == 1
                        for h in (0, 1):
                            hp = h * DH
                            lhsT = k2_sb[hp : hp + DH, :, b, c * P : (c + 1) * P]
                            if odd_diag:
                                q8b = (
                                    q8_sb[hp : hp + DH, b, sq * SQ + P : sq * SQ + SQ]
                                    .unsqueeze(1)
                                    .broadcast_to([DH, 2, P])
                                )
                                dst = st[h][:, j * SQ + P : (j + 1) * SQ]
                            else:
                                q8b = (
                                    q8_sb[hp : hp + DH, b, sq * SQ : sq * SQ + SQ]
                                    .unsqueeze(1)
                                    .broadcast_to([DH, 2, SQ])
                                )
                                dst = st[h][:, j * SQ : (j + 1) * SQ]
                            nc.tensor.matmul(
                                dst,
                                lhsT,
                                q8b,
                                start=(j == 0),
                                stop=(j == 1) and not diag,
                                perf_mode=DR,
                            )
                    if diag:
                        # triangular -1536 mask: even chunk's first 128
                        # queries, odd chunk's (valid) last 128 queries
                        for h in (0, 1):
                            nc.tensor.matmul(
                                st[h][:, 0:P],
                                i8_sb[:],
                                m8_sb[:],
                                start=False,
                                stop=False,
                                perf_mode=DR,
                            )
                            nc.tensor.matmul(
                                st[h][:, SQ + P : 2 * SQ],
                                i8_sb[:],
                                m8_sb[:],
                                start=False,
                                stop=True,
                                perf_mode=DR,
                            )
                    for h in (0, 1):
                        nc.scalar.activation(
                            pt[h][:, g : g + 2, :],
                            st[h][:, 0 : 2 * SQ],
                            mybir.ActivationFunctionType.Exp,
                            scale=0.125,
                        )
                    weave()
                return pt

            # ---- output work for one superblock: 4 filler units ----
            def make_output_units(b, sq, pt):
                nk = 2 * sq + 2
                held = {}

                def pv_half(hs):
                    def f():
                        if hs == 0:
                            held["pvt"] = ps_sm.tile(
                                [P, 4 * 65], F32, name="pvt", tag="sm"
                            )
                        pvt = held["pvt"]
                        for qh in (0, 1):
                            off = (2 * hs + qh) * 65
                            nck = nk - 1 if qh == 0 else nk
                            for c in range(nck):
                                nc.tensor.matmul(
                                    pvt[:, off : off + 65],
                                    pt[hs][:, c, qh * P : (qh + 1) * P],
                                    V2[:, b, c, hs * 65 : hs * 65 + 65],
                                    start=(hs == 0 and qh == 0 and c == 0),
                                    stop=(c == nck - 1),
                                )

                    return f

                def normtp():
                    pvt = held["pvt"]
                    osbs = []
                    for h in (0, 1):
                        for qh in (0, 1):
                            off = (2 * h + qh) * 65
                            r = rcp.tile([P, 1], F32, name="rr", tag="rr")
                            nc.vector.reciprocal(r[:], pvt[:, off : off + 1])
                            osb = osml.tile([P, DH], BF, name="osb")
                            nc.vector.tensor_scalar_mul(
                                osb[:], pvt[:, off + 1 : off + 65], r[:]
                            )
                            osbs.append((h, qh, osb))
                    tp = ps_sm.tile([P, SQ], BF, name="tp", tag="sm")
                    for h, qh, osb in osbs:
                        hp = h * DH
                        nc.tensor.matmul(
                            tp[hp : hp + DH, qh * P : (qh + 1) * P],
                            osb[:],
                            ident[:],
                            is_transpose=True,
                            tile_position=(0, hp),
                            start=(qh == 0),
                            stop=(qh == 1),
                        )
                    nc.vector.tensor_copy(
                        attn_oT[:, b * T + sq * SQ : b * T + (sq + 1) * SQ], tp[:]
                    )

                def proj():
                    ys = ystage.tile([P, 2, D], BF, name="ys")
                    for half in range(2):
                        tt = b * (T // P) + 2 * sq + half
                        for nh in range(2):
                            psp = ps_pj.tile([P, 512], F32, name="psp", tag="pj")
                            nc.tensor.matmul(
                                psp[:],
                                attn_oT[:, tt * P : (tt + 1) * P],
                                wp_sb[:, nh * 512 : (nh + 1) * 512],
                                start=True,
                                stop=True,
                            )
                            nc.vector.tensor_copy(
                                ys[:, half, nh * 512 : (nh + 1) * 512], psp[:]
                            )
                    t0 = (b * (T // P) + 2 * sq) * P
                    nc.sync.dma_start(
                        y_out[t0 : t0 + 2 * P, :].rearrange(
                            "(s p) d -> p s d", s=2
                        ),
                        ys[:],
                    )

                return [pv_half(0), pv_half(1), normtp, proj]

            # ---- main schedule ----
            blocks = [(0, s) for s in range(8)] + [(1, s) for s in range(8)]
            # enqueue tc j's pieces after block idx (key); deadline = block
            # that first reads tc j's q/k
            tc_after = {0: 1, 1: 2, 3: 3, 5: 4, 7: 5, 9: 6, 11: 7}
            tc_deadline = {1: 2, 2: 4, 3: 6, 4: 8, 5: 10, 6: 12, 7: 14}

            for piece in make_tc_pieces(0):
                piece()
            pend = {}
            for i, (b, sq) in enumerate(blocks):
                force_drain(i)
                pt = emit_scores(b, sq)
                pend[i] = (b, sq, pt)
                if i - 2 in pend:
                    bb, ss, pp = pend.pop(i - 2)
                    for u in make_output_units(bb, ss, pp):
                        filler.append((i + 2, u))
                if i in tc_after:
                    j = tc_after[i]
                    for piece in make_tc_pieces(j):
                        filler.append((tc_deadline[j], piece))
            for i in sorted(pend):
                bb, ss, pp = pend[i]
                for u in make_output_units(bb, ss, pp):
                    filler.append((99, u))
            pop_fillers(len(filler))

    nc.compile()
    return nc


def get_nc():
    global _CACHED_NC
    if _CACHED_NC is None:
        _CACHED_NC = build_nc()
    return _CACHED_NC


def make_in_maps(x, w_qkv, b_qkv, w_proj):
    bf = ml_dtypes.bfloat16
    e5 = ml_dtypes.float8_e5m2
    x = np.asarray(x, dtype=np.float32).reshape(TA, D)
    w_qkv = np.asarray(w_qkv, dtype=np.float32)
    b_qkv = np.asarray(b_qkv, dtype=np.float32)
    w_proj = np.asarray(w_proj, dtype=np.float32)
    xT = np.ascontiguousarray(x.T).astype(bf)  # [D, TA] bf16, replicated
    # fp8e5 identity + strict-upper-triangular additive mask (-1536); the
    # same [128,128] triangle serves both diagonal chunks. Slot 1 of each
    # DoubleRow pair is zeros.
    i8 = np.zeros((P, 2, P), dtype=e5)
    i8[:, 0, :] = np.eye(P, dtype=np.float32).astype(e5)
    m8 = np.zeros((P, 2, P), dtype=e5)
    kk = np.arange(P)[:, None]
    qq = np.arange(P)[None, :]
    m8[:, 0, :] = np.where(kk > qq, -1536.0, 0.0).astype(e5)
    in_maps = []
    for c in range(N_CORES):
        lo = 2 * c * DH  # first feature column of this core's 2 heads
        in_maps.append(
            {
                "xT": xT,
                "wq": np.ascontiguousarray(w_qkv[:, lo : lo + P]).astype(bf),
                "wk": np.ascontiguousarray(w_qkv[:, D + lo : D + lo + P]).astype(bf),
                "wv": np.ascontiguousarray(
                    w_qkv[:, 2 * D + lo : 2 * D + lo + P]
                ).astype(bf),
                "bq": np.ascontiguousarray(b_qkv[lo : lo + P][:, None]),
                "bk": np.ascontiguousarray(b_qkv[D + lo : D + lo + P][:, None]),
                "wp": np.ascontiguousarray(w_proj[lo : lo + P, :]).astype(bf),
                "i8": i8,
                "m8": m8,
            }
        )
    return in_maps


def gather(results, b_qkv, w_proj, b_proj):
    b_qkv = np.asarray(b_qkv, dtype=np.float32)
    w_proj = np.asarray(w_proj, dtype=np.float32)
    b_proj = np.asarray(b_proj, dtype=np.float32)
    y = np.zeros((TA, D), dtype=np.float32)
    for c in range(N_CORES):
        y += np.asarray(results[c]["y"], dtype=np.float32)
    # exact host-side fold of the v-bias and projection bias:
    # softmax rows sum to 1, so the v-bias passes through attention intact.
    y += b_qkv[2 * D : 3 * D] @ w_proj + b_proj
    return y.reshape(B, T, D)


def run(x, w_qkv, b_qkv, w_proj, b_proj, trace=False, **spmd_kwargs):
    nc = get_nc()
    in_maps = make_in_maps(x, w_qkv, b_qkv, w_proj)
    res = run_bass_kernel_spmd(
        nc, in_maps, list(range(N_CORES)), trace=trace, **spmd_kwargs
    )
    return gather(res.results, b_qkv, w_proj, b_proj), res


def kernel(x, w_qkv, b_qkv, w_proj, b_proj):
    y, _ = run(x, w_qkv, b_qkv, w_proj, b_proj)
    return y


# revision 28
# speedup vs baseline: 1.2274x; 1.0471x over previous
"""Multi-head causal self-attention (B=2, T=2048, D=1024, H=16, Dh=64) on 8
Trainium2 NeuronCores.

Sharding (Megatron-style tensor parallel over heads):
  - Each core owns 2 heads (core c -> heads 2c, 2c+1) for both batch rows.
  - w_qkv column-sharded per core ([1024, 128] per q/k/v, bf16 on host).
  - w_proj row-sharded ([128, 1024] bf16); cores emit partial projection
    outputs which the host sums (plus bias terms folded exactly on host).
  - x replicated, passed pre-transposed AND pre-cast: xT [1024, 4096] bf16
    (halves the input DMA vs fp32 and removes the on-device cast).

Device-side per core:
  qT/kT/vT = W^T x^T on PE. q evicted as fp8e4 (q8); k as an fp8e4 pair
  (k8, k8r) slot-interleaved in one tile, k8r = (k + bk) - k8 being the
  quantization residual. Scores are fp8 DoubleRow matmuls (0.5 cyc/col):
  stationary tiles (k8, k8r), moving tiles (q8, q8) via a stride-0
  broadcast AP -> (k8+k8r)^T q8: K accurate to ~13 bits, Q to e4m3, logit
  error ~2.5% of a 0.41-std logit => ~1.2e-2 final rel err, inside 2e-2.
  Causal masks for the two diagonal chunks fold in as fp8e5
  identity.T @ (-1536 triangle) DoubleRow accumulates; the odd-diagonal
  chunk computes only its valid 128-query half and PV skips it for the
  first query sub-block. exp on ACT per 2-chunk PSUM bank; PV in bf16
  with a ones-column in V2 producing softmax denominators in col 0;
  normalize on DVE; 4 PE transposes packed per PSUM bank (tile_position
  places head 1 at partitions 64..127) + one DVE evict; projection per
  256-row block with a single batched y DMA.

Schedule: ACT (exp) is the second-busiest engine (~88us) after PE
(~93us) and exp backlog is capped by 3 score PSUM banks (~1.8us), so any
contiguous >2us stretch of non-score PE work starves ACT. The emitter
therefore weaves: score groups are emitted back-to-back while qkv chains
(split into ~0.9us pieces) and block output work (PV/normalize/proj,
pipelined >=2 blocks behind) are popped from a filler queue between
groups at a rate proportional to the remaining filler/group ratio.
Forced drains keep feasibility: tc j before the blocks that read it, and
output units before their pt pool slots are reused (4-block window).
"""

import numpy as np
import ml_dtypes

import concourse.bacc as bacc
import concourse.bass as bass
import concourse.mybir as mybir
import concourse.tile as tile
from concourse.bass_utils import run_bass_kernel_spmd
from concourse.masks import make_identity

N_CORES = 8
B = 2
T = 2048
D = 1024
H = 16
DH = 64
TA = B * T  # 4096 rows total
P = 128
NQB = T // P  # 16 key chunks per batch
KC = D // P  # 8 contraction chunks for qkv
SQ = 256  # superblock query count
BF = mybir.dt.bfloat16
F32 = mybir.dt.float32
F8 = mybir.dt.float8e4
F8E5 = mybir.dt.float8e5
DR = mybir.MatmulPerfMode.DoubleRow
ADD = mybir.AluOpType.add
SUB = mybir.AluOpType.subtract

_CACHED_NC = None


def build_nc():
    """Build the per-core Bass program (identical on all 8 cores)."""
    nc = bacc.Bacc("TRN2", target_bir_lowering=False, debug=False, num_devices=N_CORES)

    xT_in = nc.dram_tensor("xT", [D, TA], BF, kind="ExternalInput").ap()
    wq_in = nc.dram_tensor("wq", [D, P], BF, kind="ExternalInput").ap()
    wk_in = nc.dram_tensor("wk", [D, P], BF, kind="ExternalInput").ap()
    wv_in = nc.dram_tensor("wv", [D, P], BF, kind="ExternalInput").ap()
    bq_in = nc.dram_tensor("bq", [P, 1], F32, kind="ExternalInput").ap()
    bk_in = nc.dram_tensor("bk", [P, 1], F32, kind="ExternalInput").ap()
    wp_in = nc.dram_tensor("wp", [P, D], BF, kind="ExternalInput").ap()
    i8_in = nc.dram_tensor("i8", [P, 2, P], F8E5, kind="ExternalInput").ap()
    m8_in = nc.dram_tensor("m8", [P, 2, P], F8E5, kind="ExternalInput").ap()
    y_out = nc.dram_tensor("y", [TA, D], BF, kind="ExternalOutput").ap()

    with tile.TileContext(nc) as tc:
        with (
            tc.tile_pool(name="const", bufs=1) as const,
            tc.tile_pool(name="xts", bufs=1) as xts,
            tc.tile_pool(name="qkv", bufs=1) as qkv,
            tc.tile_pool(name="ptp", bufs=8) as ptp,
            tc.tile_pool(name="osml", bufs=8) as osml,
            tc.tile_pool(name="rcp", bufs=8) as rcp,
            tc.tile_pool(name="ystage", bufs=3) as ystage,
            tc.tile_pool(name="ps_st", bufs=3, space="PSUM") as ps_st,
            tc.tile_pool(name="ps_qk", bufs=2, space="PSUM") as ps_qk,
            tc.tile_pool(name="ps_pj", bufs=2, space="PSUM") as ps_pj,
            tc.tile_pool(name="ps_sm", bufs=1, space="PSUM") as ps_sm,
        ):
            # ---- constants ----
            ident = const.tile([P, P], BF)
            make_identity(nc, ident[:])
            bq_sb = const.tile([P, 1], F32)
            nc.sync.dma_start(bq_sb[:], bq_in[:])
            bk_sb = const.tile([P, 1], F32)
            nc.sync.dma_start(bk_sb[:], bk_in[:])
            # touch Exp now so the ACT table load (1.3us) is off the
            # critical path of the first real exp
            warm_act = const.tile([P, 1], BF)
            nc.scalar.activation(
                warm_act[:], bq_sb[:], mybir.ActivationFunctionType.Exp
            )

            # ---- xT load (bf16 from host): one strided DMA per 512-col
            # T-chunk ([128, 8, 512] = all contraction chunks at once) so
            # tc j is runnable after j+1 DMAs; weight DMAs woven between
            # the first x chunks in need-order ----
            xT_sb = xts.tile([P, KC, TA], BF)
            xT_r = xT_in.rearrange("(c p) t -> p c t", c=KC)
            i8_sb = const.tile([P, 2, P], F8E5)
            m8_sb = const.tile([P, 2, P], F8E5)
            wp_sb = const.tile([P, D], BF)
            w_sb = {}
            for name in ("q", "k", "v"):
                w_sb[name] = const.tile([P, KC, P], BF, name=f"w{name}sb")
            for s in range(TA // 512):
                if s == 0:  # split so the first qkv chain starts sooner
                    nc.sync.dma_start(
                        xT_sb[:, 0:4, 0:512], xT_r[:, 0:4, 0:512]
                    )
                    nc.sync.dma_start(
                        xT_sb[:, 4:KC, 0:512], xT_r[:, 4:KC, 0:512]
                    )
                else:
                    nc.sync.dma_start(
                        xT_sb[:, :, s * 512 : (s + 1) * 512],
                        xT_r[:, :, s * 512 : (s + 1) * 512],
                    )
                if s == 0:
                    # weights ride the scalar queue so they don't delay x
                    for name, ap in (("q", wq_in), ("k", wk_in), ("v", wv_in)):
                        nc.scalar.dma_start(
                            w_sb[name][:], ap.rearrange("(c p) f -> p c f", c=KC)
                        )
                if s == 1:  # needed later than qkv; keep off the startup queue
                    nc.scalar.dma_start(i8_sb[:], i8_in[:])
                    nc.scalar.dma_start(m8_sb[:], m8_in[:])
                    nc.scalar.dma_start(wp_sb[:], wp_in[:])

            # ---- PE warmup: dependency-free matmuls cover the x-load ramp
            # so HAM reaches 2.4 GHz before the first real qkv matmul ----
            wm = ps_pj.tile([P, 512], F32, name="warm", tag="pj")
            for _ in range(80):
                nc.tensor.matmul(
                    wm[:, 0:P], ident[:], ident[:], start=True, stop=True
                )

            # ---- persistent activation tiles ----
            q8_sb = qkv.tile([P, B, T], F8)
            k2_sb = qkv.tile([P, 2, B, T], F8)
            vT_sb = qkv.tile([P, B, T], BF)
            # V2 per (b, key-chunk): [1 | V_h0 (64) | 1 | V_h1 (64)]
            V2 = qkv.tile([P, B, NQB, 130], BF)
            nc.vector.memset(V2[:, :, :, 0], 1.0)
            nc.vector.memset(V2[:, :, :, 65], 1.0)
            attn_oT = qkv.tile([P, TA], BF)

            # ---- qkv T-chunk as 7 filler pieces (~0.9us PE each) ----
            def make_tc_pieces(tcg):
                b = tcg // 4
                col = (tcg % 4) * 512
                held = {}

                def chain_a(blk):
                    def f():
                        pst = ps_qk.tile([P, 512], F32, name="pqk", tag="qk")
                        held[blk] = pst
                        for c in range(4):
                            nc.tensor.matmul(
                                pst[:],
                                w_sb[blk][:, c, :],
                                xT_sb[:, c, tcg * 512 : (tcg + 1) * 512],
                                start=(c == 0),
                                stop=False,
                            )

                    return f

                def chain_b(blk):
                    def f():
                        pst = held[blk]
                        for c in range(4, KC):
                            nc.tensor.matmul(
                                pst[:],
                                w_sb[blk][:, c, :],
                                xT_sb[:, c, tcg * 512 : (tcg + 1) * 512],
                                start=False,
                                stop=(c == KC - 1),
                            )
                        if blk == "q":
                            nc.vector.tensor_scalar(
                                q8_sb[:, b, col : col + 512],
                                pst[:],
                                bq_sb[:],
                                None,
                                op0=ADD,
                            )
                        elif blk == "k":
                            k8 = k2_sb[:, 0, b, col : col + 512]
                            nc.vector.tensor_scalar(
                                k8, pst[:], bk_sb[:], None, op0=ADD
                            )
                            nc.vector.scalar_tensor_tensor(
                                k2_sb[:, 1, b, col : col + 512],
                                pst[:],
                                bk_sb[:],
                                k8,
                                op0=ADD,
                                op1=SUB,
                            )
                        else:
                            nc.vector.tensor_copy(
                                vT_sb[:, b, col : col + 512], pst[:]
                            )

                    return f

                def vfix():
                    bs = (tcg % 4) * 4
                    vtp = ps_qk.tile([P, 4, P], BF, name="vtp", tag="qk")
                    for j in range(4):
                        s = bs + j
                        nc.tensor.matmul(
                            vtp[:, j, :],
                            vT_sb[:, b, s * P : (s + 1) * P],
                            ident[:],
                            is_transpose=True,
                            start=(j == 0),
                            stop=(j == 3),
                        )
                    nc.vector.tensor_copy(
                        V2[:, b, bs : bs + 4, :].rearrange(
                            "p s (h x) -> p s h x", h=2
                        )[:, :, :, 1:65],
                        vtp[:].rearrange("p s (h d) -> p s h d", h=2),
                    )

                # q_a/k_a consume the first x half while the second arrives
                return [
                    chain_a("q"),
                    chain_a("k"),
                    chain_b("q"),
                    chain_a("v"),
                    chain_b("k"),
                    chain_b("v"),
                    vfix,
                ]

            # ---- filler queue: (deadline_position, closure) FIFO ----
            filler = []
            groups_done = [0]

            def pop_fillers(k):
                for _ in range(min(k, len(filler))):
                    filler.pop(0)[1]()

            def force_drain(upto):
                due = [u for u in filler if u[0] <= upto]
                filler[:] = [u for u in filler if u[0] > upto]
                for _, fn in due:
                    fn()

            def weave():
                groups_done[0] += 1
                if not filler:
                    return
                left = max(groups_total - groups_done[0], 1)
                k = -(-len(filler) // left)  # global even spread
                # urgency: items due at the earliest deadline must drain
                # evenly over the groups remaining before that position
                d0 = min(f[0] for f in filler)
                n_due = sum(1 for f in filler if f[0] <= d0)
                gb = (Gpre[d0] if d0 < len(Gpre) else groups_total) - groups_done[0]
                k = max(k, n_due if gb <= 0 else -(-n_due // gb))
                pop_fillers(min(k, 8))

            # ---- scores for one 256-query superblock (fp8 DoubleRow) ----
            def emit_scores(b, sq):
                nk = 2 * sq + 2
                q0 = b * T + sq * SQ
                pt = {}
                for h in (0, 1):
                    pt[h] = ptp.tile([P, NQB, SQ], BF, name="ptt", tag="pt")
                for g in range(0, nk, 2):
                    diag = g == nk - 2
                    st = {}
                    for h in (0, 1):
                        st[h] = ps_st.tile([P, 512], F32, name="st", tag="st")
                    for j in (0, 1):
                        c = g + j
                        odd_diag = diag and j == 1
                        for h in (0, 1):
                            hp = h * DH
                            lhsT = k2_sb[hp : hp + DH, :, b, c * P : (c + 1) * P]
                            if odd_diag:
                                q8b = (
                                    q8_sb[hp : hp + DH, b, sq * SQ + P : sq * SQ + SQ]
                                    .unsqueeze(1)
                                    .broadcast_to([DH, 2, P])
                                )
                                dst = st[h][:, j * SQ + P : (j + 1) * SQ]
                            else:
                                q8b = (
                                    q8_sb[hp : hp + DH, b, sq * SQ : sq * SQ + SQ]
                                    .unsqueeze(1)
                                    .broadcast_to([DH, 2, SQ])
                                )
                                dst = st[h][:, j * SQ : (j + 1) * SQ]
                            nc.tensor.matmul(
                                dst,
                                lhsT,
                                q8b,
                                start=(j == 0),
                                stop=(j == 1) and not diag,
                                perf_mode=DR,
                            )
                    if diag:
                        # triangular -1536 mask: even chunk's first 128
                        # queries, odd chunk's (valid) last 128 queries
                        for h in (0, 1):
                            nc.tensor.matmul(
                                st[h][:, 0:P],
                                i8_sb[:],
                                m8_sb[:],
                                start=False,
                                stop=False,
                                perf_mode=DR,
                            )
                            nc.tensor.matmul(
                                st[h][:, SQ + P : 2 * SQ],
                                i8_sb[:],
                                m8_sb[:],
                                start=False,
                                stop=True,
                                perf_mode=DR,
                            )
                    for h in (0, 1):
                        nc.scalar.activation(
                            pt[h][:, g : g + 2, :],
                            st[h][:, 0 : 2 * SQ],
                            mybir.ActivationFunctionType.Exp,
                            scale=0.125,
                        )
                    weave()
                return pt

            # ---- output work for one superblock: 4 filler units ----
            def make_output_units(b, sq, pt, pool=None, ptag=None, split_dma=False):
                nk = 2 * sq + 2
                held = {}
                pool = pool or ps_sm
                ptag = ptag or "sm"

                def pv_half(hs):
                    def f():
                        if hs == 0:
                            held["pvt"] = pool.tile(
                                [P, 4 * 65], F32, name="pvt", tag=ptag
                            )
                        pvt = held["pvt"]
                        for qh in (0, 1):
                            off = (2 * hs + qh) * 65
                            nck = nk - 1 if qh == 0 else nk
                            for c in range(nck):
                                nc.tensor.matmul(
                                    pvt[:, off : off + 65],
                                    pt[hs][:, c, qh * P : (qh + 1) * P],
                                    V2[:, b, c, hs * 65 : hs * 65 + 65],
                                    start=(hs == 0 and qh == 0 and c == 0),
                                    stop=(c == nck - 1),
                                )

                    return f

                def norm():
                    pvt = held["pvt"]
                    osbs = []
                    for h in (0, 1):
                        for qh in (0, 1):
                            off = (2 * h + qh) * 65
                            r = rcp.tile([P, 1], F32, name="rr", tag="rr")
                            nc.vector.reciprocal(r[:], pvt[:, off : off + 1])
                            osb = osml.tile([P, DH], BF, name="osb")
                            nc.vector.tensor_scalar_mul(
                                osb[:], pvt[:, off + 1 : off + 65], r[:]
                            )
                            osbs.append((h, qh, osb))
                    held["osbs"] = osbs

                def tpev():
                    tp = pool.tile([P, SQ], BF, name="tp", tag=ptag)
                    for h, qh, osb in held["osbs"]:
                        hp = h * DH
                        nc.tensor.matmul(
                            tp[hp : hp + DH, qh * P : (qh + 1) * P],
                            osb[:],
                            ident[:],
                            is_transpose=True,
                            tile_position=(0, hp),
                            start=(qh == 0),
                            stop=(qh == 1),
                        )
                    nc.vector.tensor_copy(
                        attn_oT[:, b * T + sq * SQ : b * T + (sq + 1) * SQ], tp[:]
                    )

                def proj():
                    ys = ystage.tile([P, 2, D], BF, name="ys")
                    for half in range(2):
                        tt = b * (T // P) + 2 * sq + half
                        for nh in range(2):
                            psp = ps_pj.tile([P, 512], F32, name="psp", tag="pj")
                            nc.tensor.matmul(
                                psp[:],
                                attn_oT[:, tt * P : (tt + 1) * P],
                                wp_sb[:, nh * 512 : (nh + 1) * 512],
                                start=True,
                                stop=True,
                            )
                            nc.vector.tensor_copy(
                                ys[:, half, nh * 512 : (nh + 1) * 512], psp[:]
                            )
                        if split_dma:
                            nc.sync.dma_start(
                                y_out[tt * P : (tt + 1) * P, :], ys[:, half, :]
                            )
                    if not split_dma:
                        t0 = (b * (T // P) + 2 * sq) * P
                        nc.sync.dma_start(
                            y_out[t0 : t0 + 2 * P, :].rearrange(
                                "(s p) d -> p s d", s=2
                            ),
                            ys[:],
                        )

                return [pv_half(0), pv_half(1), norm, tpev, proj]

            # ---- main schedule ----
            # Emission order interleaves batch 1's small blocks into late
            # batch 0 so the ACT (exp) feed never thins out at the batch
            # boundary; positions are indices into this order.
            blocks = [
                (0, 0), (0, 1), (0, 2), (0, 3), (0, 4), (0, 5),
                (1, 0), (0, 6), (1, 1), (0, 7),
                (1, 2), (1, 3), (1, 7), (1, 6), (1, 5), (1, 4),
            ]
            Gpre = []
            acc = 0
            for _, ss in blocks:
                Gpre.append(acc)
                acc += ss + 1
            groups_total = acc
            # tc j's pieces enqueue after position p, force-drain before the
            # first position whose block reads tc j's q/k
            tc_after = {0: 1, 1: 2, 2: 3, 3: 4, 5: 5, 7: 6, 9: 7}
            tc_deadline = {1: 2, 2: 4, 3: 6, 4: 6, 5: 10, 6: 12, 7: 12}

            for piece in make_tc_pieces(0):
                piece()
            pend = {}
            for i, (b, sq) in enumerate(blocks):
                force_drain(i)
                pt = emit_scores(b, sq)
                pend[i] = (b, sq, pt)
                if i - 2 in pend:
                    bb, ss, pp = pend.pop(i - 2)
                    for u in make_output_units(bb, ss, pp):
                        filler.append((i + 2, u))
                if i in tc_after:
                    j = tc_after[i]
                    for piece in make_tc_pieces(j):
                        filler.append((tc_deadline[j], piece))
            # tail: interleave the last two blocks' stages so PE and DVE
            # overlap instead of serializing cross-engine round-trips. The
            # last block's PSUM tiles come from the (now idle) score banks
            # so the ps_sm single-slot ring doesn't force serialization,
            # and its y DMAs are split per 128-row tile.
            pop_fillers(len(filler))
            (b14, s14, p14), (b15, s15, p15) = pend[14], pend[15]
            u14 = make_output_units(b14, s14, p14, split_dma=True)
            u15 = make_output_units(
                b15, s15, p15, pool=ps_st, ptag="st", split_dma=True
            )
            for fn in (u14[0], u14[1], u14[2], u15[0], u15[1], u14[3],
                       u14[4], u15[2], u15[3], u15[4]):
                fn()

    nc.compile()
    return nc


def get_nc():
    global _CACHED_NC
    if _CACHED_NC is None:
        _CACHED_NC = build_nc()
    return _CACHED_NC


def make_in_maps(x, w_qkv, b_qkv, w_proj):
    bf = ml_dtypes.bfloat16
    e5 = ml_dtypes.float8_e5m2
    x = np.asarray(x, dtype=np.float32).reshape(TA, D)
    w_qkv = np.asarray(w_qkv, dtype=np.float32)
    b_qkv = np.asarray(b_qkv, dtype=np.float32)
    w_proj = np.asarray(w_proj, dtype=np.float32)
    xT = np.ascontiguousarray(x.T).astype(bf)  # [D, TA] bf16, replicated
    # fp8e5 identity + strict-upper-triangular additive mask (-1536); the
    # same [128,128] triangle serves both diagonal chunks. Slot 1 of each
    # DoubleRow pair is zeros.
    i8 = np.zeros((P, 2, P), dtype=e5)
    i8[:, 0, :] = np.eye(P, dtype=np.float32).astype(e5)
    m8 = np.zeros((P, 2, P), dtype=e5)
    kk = np.arange(P)[:, None]
    qq = np.arange(P)[None, :]
    m8[:, 0, :] = np.where(kk > qq, -1536.0, 0.0).astype(e5)
    in_maps = []
    for c in range(N_CORES):
        lo = 2 * c * DH  # first feature column of this core's 2 heads
        in_maps.append(
            {
                "xT": xT,
                "wq": np.ascontiguousarray(w_qkv[:, lo : lo + P]).astype(bf),
                "wk": np.ascontiguousarray(w_qkv[:, D + lo : D + lo + P]).astype(bf),
                "wv": np.ascontiguousarray(
                    w_qkv[:, 2 * D + lo : 2 * D + lo + P]
                ).astype(bf),
                "bq": np.ascontiguousarray(b_qkv[lo : lo + P][:, None]),
                "bk": np.ascontiguousarray(b_qkv[D + lo : D + lo + P][:, None]),
                "wp": np.ascontiguousarray(w_proj[lo : lo + P, :]).astype(bf),
                "i8": i8,
                "m8": m8,
            }
        )
    return in_maps


def gather(results, b_qkv, w_proj, b_proj):
    b_qkv = np.asarray(b_qkv, dtype=np.float32)
    w_proj = np.asarray(w_proj, dtype=np.float32)
    b_proj = np.asarray(b_proj, dtype=np.float32)
    y = np.zeros((TA, D), dtype=np.float32)
    for c in range(N_CORES):
        y += np.asarray(results[c]["y"], dtype=np.float32)
    # exact host-side fold of the v-bias and projection bias:
    # softmax rows sum to 1, so the v-bias passes through attention intact.
    y += b_qkv[2 * D : 3 * D] @ w_proj + b_proj
    return y.reshape(B, T, D)


def run(x, w_qkv, b_qkv, w_proj, b_proj, trace=False, **spmd_kwargs):
    nc = get_nc()
    in_maps = make_in_maps(x, w_qkv, b_qkv, w_proj)
    res = run_bass_kernel_spmd(
        nc, in_maps, list(range(N_CORES)), trace=trace, **spmd_kwargs
    )
    return gather(res.results, b_qkv, w_proj, b_proj), res


def kernel(x, w_qkv, b_qkv, w_proj, b_proj):
    y, _ = run(x, w_qkv, b_qkv, w_proj, b_proj)
    return y


# revision 44
# speedup vs baseline: 1.2778x; 1.0410x over previous
"""Multi-head causal self-attention (B=2, T=2048, D=1024, H=16, Dh=64) on 8
Trainium2 NeuronCores.

Sharding (Megatron-style tensor parallel over heads):
  - Each core owns 2 heads (core c -> heads 2c, 2c+1) for both batch rows.
  - w_qkv column-sharded per core ([1024, 128] per q/k/v, bf16 on host).
  - w_proj row-sharded ([128, 1024] bf16); cores emit partial projection
    outputs which the host sums (plus bias terms folded exactly on host).
  - x replicated, passed pre-transposed AND pre-cast: xT [1024, 4096] bf16
    (halves the input DMA vs fp32 and removes the on-device cast).

Device-side per core:
  qT/kT/vT = W^T x^T on PE. q evicted as fp8e4 (q8); k as an fp8e4 pair
  (k8, k8r) slot-interleaved in one tile, k8r = (k + bk) - k8 being the
  quantization residual. Scores are fp8 DoubleRow matmuls (0.5 cyc/col):
  stationary tiles (k8, k8r), moving tiles (q8, q8) via a stride-0
  broadcast AP -> (k8+k8r)^T q8: K accurate to ~13 bits, Q to e4m3, logit
  error ~2.5% of a 0.41-std logit => ~1.2e-2 final rel err, inside 2e-2.
  Causal masks for the two diagonal chunks fold in as fp8e5
  identity.T @ (-1536 triangle) DoubleRow accumulates; the odd-diagonal
  chunk computes only its valid 128-query half and PV skips it for the
  first query sub-block. exp on ACT per 2-chunk PSUM bank; PV in bf16
  with a ones-column in V2 producing softmax denominators in col 0;
  normalize on DVE; 4 PE transposes packed per PSUM bank (tile_position
  places head 1 at partitions 64..127) + one DVE evict; projection per
  256-row block with a single batched y DMA.

Schedule: ACT (exp) is the second-busiest engine (~88us) after PE
(~93us) and exp backlog is capped by 3 score PSUM banks (~1.8us), so any
contiguous >2us stretch of non-score PE work starves ACT. The emitter
therefore weaves: score groups are emitted back-to-back while qkv chains
(split into ~0.9us pieces) and block output work (PV/normalize/proj,
pipelined >=2 blocks behind) are popped from a filler queue between
groups at a rate proportional to the remaining filler/group ratio.
Forced drains keep feasibility: tc j before the blocks that read it, and
output units before their pt pool slots are reused (4-block window).
"""

import numpy as np
import ml_dtypes

import concourse.bacc as bacc
import concourse.bass as bass
import concourse.mybir as mybir
import concourse.tile as tile
from concourse.bass_utils import run_bass_kernel_spmd
from concourse.masks import make_identity

N_CORES = 8
B = 2
T = 2048
D = 1024
H = 16
DH = 64
TA = B * T  # 4096 rows total
P = 128
NQB = T // P  # 16 key chunks per batch
KC = D // P  # 8 contraction chunks for qkv
SQ = 256  # superblock query count
BF = mybir.dt.bfloat16
F32 = mybir.dt.float32
F8 = mybir.dt.float8e4
F8E5 = mybir.dt.float8e5
DR = mybir.MatmulPerfMode.DoubleRow
ADD = mybir.AluOpType.add
SUB = mybir.AluOpType.subtract
MUL = mybir.AluOpType.mult

_CACHED_NC = None


def build_nc():
    """Build the per-core Bass program (identical on all 8 cores)."""
    nc = bacc.Bacc("TRN2", target_bir_lowering=False, debug=False, num_devices=N_CORES)

    x8_in = nc.dram_tensor("x8", [D, TA], F8, kind="ExternalInput").ap()
    x8r_in = nc.dram_tensor("x8r", [D, TA], F8, kind="ExternalInput").ap()
    wq_in = nc.dram_tensor("wq", [D, 2, P], F8, kind="ExternalInput").ap()
    wk_in = nc.dram_tensor("wk", [D, 2, P], F8, kind="ExternalInput").ap()
    wv_in = nc.dram_tensor("wv", [D, 2, P], F8, kind="ExternalInput").ap()
    bq_in = nc.dram_tensor("bq", [P, 1], F32, kind="ExternalInput").ap()
    bk_in = nc.dram_tensor("bk", [P, 1], F32, kind="ExternalInput").ap()
    wp_in = nc.dram_tensor("wp", [P, D], BF, kind="ExternalInput").ap()
    i8_in = nc.dram_tensor("i8", [P, 2, P], F8E5, kind="ExternalInput").ap()
    m8_in = nc.dram_tensor("m8", [P, 2, P], F8E5, kind="ExternalInput").ap()
    y_out = nc.dram_tensor("y", [TA, D], BF, kind="ExternalOutput").ap()

    with tile.TileContext(nc) as tc:
        with (
            tc.tile_pool(name="const", bufs=1) as const,
            tc.tile_pool(name="xts", bufs=1) as xts,
            tc.tile_pool(name="qkv", bufs=1) as qkv,
            tc.tile_pool(name="ptp", bufs=8) as ptp,
            tc.tile_pool(name="osml", bufs=8) as osml,
            tc.tile_pool(name="rcp", bufs=8) as rcp,
            tc.tile_pool(name="ystage", bufs=3) as ystage,
            tc.tile_pool(name="ps_st", bufs=3, space="PSUM") as ps_st,
            tc.tile_pool(name="ps_qk", bufs=2, space="PSUM") as ps_qk,
            tc.tile_pool(name="ps_pj", bufs=2, space="PSUM") as ps_pj,
            tc.tile_pool(name="ps_sm", bufs=1, space="PSUM") as ps_sm,
        ):
            # ---- constants ----
            ident = const.tile([P, P], BF)
            make_identity(nc, ident[:])
            bq_sb = const.tile([P, 1], F32)
            nc.sync.dma_start(bq_sb[:], bq_in[:])
            bk_sb = const.tile([P, 1], F32)
            nc.sync.dma_start(bk_sb[:], bk_in[:])
            # touch Exp now so the ACT table load (1.3us) is off the
            # critical path of the first real exp
            warm_act = const.tile([P, 1], BF)
            nc.scalar.activation(
                warm_act[:], bq_sb[:], mybir.ActivationFunctionType.Exp
            )

            # ---- xT load (bf16 from host): one strided DMA per 512-col
            # T-chunk ([128, 8, 512] = all contraction chunks at once) so
            # tc j is runnable after j+1 DMAs; weight DMAs woven between
            # the first x chunks in need-order ----
            x8_sb = xts.tile([P, KC, TA], F8)
            x8r_sb = xts.tile([P, KC, TA], F8)
            x8_r = x8_in.rearrange("(c p) t -> p c t", c=KC)
            x8r_r = x8r_in.rearrange("(c p) t -> p c t", c=KC)
            i8_sb = const.tile([P, 2, P], F8E5)
            m8_sb = const.tile([P, 2, P], F8E5)
            wp_sb = const.tile([P, D], BF)
            w_sb = {}
            for name in ("q", "k", "v"):
                w_sb[name] = const.tile([P, KC, 2, P], F8, name=f"w{name}sb")
            # startup-critical DMAs in exact need-order of the first qkv
            # chains and B00's scores (masks before the first diag group)
            nc.sync.dma_start(
                w_sb["q"][:], wq_in.rearrange("(c p) s f -> p c s f", c=KC)
            )
            nc.sync.dma_start(x8_sb[:, :, 0:512], x8_r[:, :, 0:512])
            nc.sync.dma_start(
                w_sb["k"][:], wk_in.rearrange("(c p) s f -> p c s f", c=KC)
            )
            nc.sync.dma_start(x8r_sb[:, :, 0:512], x8r_r[:, :, 0:512])
            nc.sync.dma_start(
                w_sb["v"][:], wv_in.rearrange("(c p) s f -> p c s f", c=KC)
            )
            nc.sync.dma_start(i8_sb[:], i8_in[:])
            nc.sync.dma_start(m8_sb[:], m8_in[:])
            for s in range(1, TA // 512):
                nc.sync.dma_start(
                    x8_sb[:, :, s * 512 : (s + 1) * 512],
                    x8_r[:, :, s * 512 : (s + 1) * 512],
                )
                nc.sync.dma_start(
                    x8r_sb[:, :, s * 512 : (s + 1) * 512],
                    x8r_r[:, :, s * 512 : (s + 1) * 512],
                )
                if s == 1:  # needed from position 2 on; off the hot queue
                    nc.scalar.dma_start(wp_sb[:], wp_in[:])

            # ---- PE warmup: dependency-free matmuls cover the x-load ramp
            # so HAM reaches 2.4 GHz before the first real qkv matmul ----
            wm = ps_pj.tile([P, 512], F32, name="warm", tag="pj")
            for _ in range(44):
                nc.tensor.matmul(
                    wm[:, 0:P], ident[:], ident[:], start=True, stop=True
                )

            # ---- persistent activation tiles ----
            q8_sb = qkv.tile([P, B, T], F8)
            k2_sb = qkv.tile([P, 2, B, T], F8)
            vT_sb = qkv.tile([P, B, T], BF)
            # V2 per (b, key-chunk): [1 | V_h0 (64) | 1 | V_h1 (64)]
            V2 = qkv.tile([P, B, NQB, 130], BF)
            nc.vector.memset(V2[:, :, :, 0], 1.0)
            nc.vector.memset(V2[:, :, :, 65], 1.0)
            attn_oT = qkv.tile([P, TA], BF)

            # ---- qkv T-chunk as 10 filler pieces (~0.45us PE each) ----
            # q/k/v in fp8 DoubleRow with full residual compensation:
            # psum = (w8+w8r)^T x8 + w8^T x8r  (w pre-scaled 64x on host,
            # rescaled at evict; only the negligible w8r*x8r term is
            # dropped). 3072 PE cycles per [128,512] tile vs 4096 bf16.
            SW64 = 1.0 / 64.0

            def make_tc_pieces(tcg):
                b = tcg // 4
                col = (tcg % 4) * 512
                held = {}
                xs = slice(tcg * 512, (tcg + 1) * 512)

                def chain_a(blk, half):
                    def f():
                        if half == 0:
                            held[blk] = ps_qk.tile(
                                [P, 512], F32, name="pqk", tag="qk"
                            )
                        pst = held[blk]
                        for c in range(half * 4, half * 4 + 4):
                            x8b = (
                                x8_sb[:, c, xs]
                                .unsqueeze(1)
                                .broadcast_to([P, 2, 512])
                            )
                            nc.tensor.matmul(
                                pst[:],
                                w_sb[blk][:, c, :, :],
                                x8b,
                                start=(c == 0),
                                stop=False,
                                perf_mode=DR,
                            )

                    return f

                def chain_b(blk):
                    def f():
                        pst = held[blk]
                        for cp in range(0, KC, 2):
                            nc.tensor.matmul(
                                pst[:],
                                w_sb[blk][:, cp : cp + 2, 0, :],
                                x8r_sb[:, cp : cp + 2, xs],
                                start=False,
                                stop=(cp == KC - 2),
                                perf_mode=DR,
                            )
                        if blk == "q":
                            nc.vector.tensor_scalar(
                                q8_sb[:, b, col : col + 512],
                                pst[:],
                                SW64,
                                bq_sb[:],
                                op0=MUL,
                                op1=ADD,
                            )
                        elif blk == "k":
                            k8 = k2_sb[:, 0, b, col : col + 512]
                            nc.vector.tensor_scalar(
                                k8, pst[:], SW64, bk_sb[:], op0=MUL, op1=ADD
                            )
                            # residual drops bk's own quantization residual
                            # (exact when bk == 0)
                            nc.vector.scalar_tensor_tensor(
                                k2_sb[:, 1, b, col : col + 512],
                                pst[:],
                                SW64,
                                k8,
                                op0=MUL,
                                op1=SUB,
                            )
                        else:
                            nc.vector.tensor_scalar(
                                vT_sb[:, b, col : col + 512],
                                pst[:],
                                SW64,
                                None,
                                op0=MUL,
                            )

                    return f

                def vfix():
                    bs = (tcg % 4) * 4
                    vtp = ps_qk.tile([P, 4, P], BF, name="vtp", tag="qk")
                    for j in range(4):
                        s = bs + j
                        nc.tensor.matmul(
                            vtp[:, j, :],
                            vT_sb[:, b, s * P : (s + 1) * P],
                            ident[:],
                            is_transpose=True,
                            start=(j == 0),
                            stop=(j == 3),
                        )
                    nc.vector.tensor_copy(
                        V2[:, b, bs : bs + 4, :].rearrange(
                            "p s (h x) -> p s h x", h=2
                        )[:, :, :, 1:65],
                        vtp[:].rearrange("p s (h d) -> p s h d", h=2),
                    )

                return [
                    chain_a("q", 0),
                    chain_a("q", 1),
                    chain_b("q"),
                    chain_a("k", 0),
                    chain_a("k", 1),
                    chain_b("k"),
                    chain_a("v", 0),
                    chain_a("v", 1),
                    chain_b("v"),
                    vfix,
                ]

            # ---- filler queue: (deadline_position, closure) FIFO ----
            filler = []
            groups_done = [0]

            def pop_fillers(k):
                for _ in range(min(k, len(filler))):
                    filler.pop(0)[1]()

            def force_drain(upto):
                due = [u for u in filler if u[0] <= upto]
                filler[:] = [u for u in filler if u[0] > upto]
                for _, fn in due:
                    fn()

            def weave():
                groups_done[0] += 1
                if not filler:
                    return
                left = max(groups_total - groups_done[0], 1)
                k = -(-len(filler) // left)  # global even spread
                # urgency: items due at the earliest deadline must drain
                # evenly over the groups remaining before that position
                d0 = min(f[0] for f in filler)
                n_due = sum(1 for f in filler if f[0] <= d0)
                gb = (Gpre[d0] if d0 < len(Gpre) else groups_total) - groups_done[0]
                k = max(k, n_due if gb <= 0 else -(-n_due // gb))
                pop_fillers(min(k, 8))

            # ---- scores for one 256-query superblock (fp8 DoubleRow) ----
            def emit_scores(b, sq):
                nk = 2 * sq + 2
                q0 = b * T + sq * SQ
                pt = {}
                for h in (0, 1):
                    pt[h] = ptp.tile([P, NQB, SQ], BF, name="ptt", tag="pt")
                for g in range(0, nk, 2):
                    diag = g == nk - 2
                    st = {}
                    for h in (0, 1):
                        st[h] = ps_st.tile([P, 512], F32, name="st", tag="st")
                    for j in (0, 1):
                        c = g + j
                        odd_diag = diag and j == 1
                        for h in (0, 1):
                            hp = h * DH
                            lhsT = k2_sb[hp : hp + DH, :, b, c * P : (c + 1) * P]
                            if odd_diag:
                                # valid queries (second 128) land at bank
                                # cols [256:384] so exp covers [0:384]
                                # contiguously; PV reads this chunk's qh=1
                                # data at pt[.., nk-1, 0:128]
                                q8b = (
                                    q8_sb[hp : hp + DH, b, sq * SQ + P : sq * SQ + SQ]
                                    .unsqueeze(1)
                                    .broadcast_to([DH, 2, P])
                                )
                                dst = st[h][:, SQ : SQ + P]
                            else:
                                q8b = (
                                    q8_sb[hp : hp + DH, b, sq * SQ : sq * SQ + SQ]
                                    .unsqueeze(1)
                                    .broadcast_to([DH, 2, SQ])
                                )
                                dst = st[h][:, j * SQ : (j + 1) * SQ]
                            nc.tensor.matmul(
                                dst,
                                lhsT,
                                q8b,
                                start=(j == 0),
                                stop=(j == 1) and not diag,
                                perf_mode=DR,
                            )
                    if diag:
                        # triangular -1536 mask: even chunk's first 128
                        # queries, odd chunk's (valid) last 128 queries
                        for h in (0, 1):
                            nc.tensor.matmul(
                                st[h][:, 0:P],
                                i8_sb[:],
                                m8_sb[:],
                                start=False,
                                stop=False,
                                perf_mode=DR,
                            )
                            nc.tensor.matmul(
                                st[h][:, SQ : SQ + P],
                                i8_sb[:],
                                m8_sb[:],
                                start=False,
                                stop=True,
                                perf_mode=DR,
                            )
                    ecols = SQ + P if diag else 2 * SQ
                    for h in (0, 1):
                        nc.scalar.activation(
                            pt[h][:, g : g + 2, :].rearrange("p a b -> p (a b)")[
                                :, 0:ecols
                            ],
                            st[h][:, 0:ecols],
                            mybir.ActivationFunctionType.Exp,
                            scale=0.125,
                        )
                    weave()
                return pt

            # ---- output work for one superblock: 4 filler units ----
            def make_output_units(b, sq, pt, pool=None, ptag=None, split_dma=False):
                nk = 2 * sq + 2
                held = {}
                pool = pool or ps_sm
                ptag = ptag or "sm"

                def pv_half(hs):
                    def f():
                        if hs == 0:
                            held["pvt"] = pool.tile(
                                [P, 4 * 65], F32, name="pvt", tag=ptag
                            )
                        pvt = held["pvt"]
                        for qh in (0, 1):
                            off = (2 * hs + qh) * 65
                            nck = nk - 1 if qh == 0 else nk
                            for c in range(nck):
                                # odd-diag chunk's valid (qh=1) data is
                                # stored at cols 0:128 (exp trim)
                                qoff = 0 if (qh == 1 and c == nk - 1) else qh * P
                                nc.tensor.matmul(
                                    pvt[:, off : off + 65],
                                    pt[hs][:, c, qoff : qoff + P],
                                    V2[:, b, c, hs * 65 : hs * 65 + 65],
                                    start=(hs == 0 and qh == 0 and c == 0),
                                    stop=(c == nck - 1),
                                )

                    return f

                def norm():
                    pvt = held["pvt"]
                    osbs = []
                    for h in (0, 1):
                        for qh in (0, 1):
                            off = (2 * h + qh) * 65
                            r = rcp.tile([P, 1], F32, name="rr", tag="rr")
                            nc.vector.reciprocal(r[:], pvt[:, off : off + 1])
                            osb = osml.tile([P, DH], BF, name="osb")
                            nc.vector.tensor_scalar_mul(
                                osb[:], pvt[:, off + 1 : off + 65], r[:]
                            )
                            osbs.append((h, qh, osb))
                    held["osbs"] = osbs

                def tpev():
                    tp = pool.tile([P, SQ], BF, name="tp", tag=ptag)
                    for h, qh, osb in held["osbs"]:
                        hp = h * DH
                        nc.tensor.matmul(
                            tp[hp : hp + DH, qh * P : (qh + 1) * P],
                            osb[:],
                            ident[:],
                            is_transpose=True,
                            tile_position=(0, hp),
                            start=(qh == 0),
                            stop=(qh == 1),
                        )
                    nc.vector.tensor_copy(
                        attn_oT[:, b * T + sq * SQ : b * T + (sq + 1) * SQ], tp[:]
                    )

                def proj():
                    ys = ystage.tile([P, 2, D], BF, name="ys")
                    for half in range(2):
                        tt = b * (T // P) + 2 * sq + half
                        for nh in range(2):
                            psp = ps_pj.tile([P, 512], F32, name="psp", tag="pj")
                            nc.tensor.matmul(
                                psp[:],
                                attn_oT[:, tt * P : (tt + 1) * P],
                                wp_sb[:, nh * 512 : (nh + 1) * 512],
                                start=True,
                                stop=True,
                            )
                            nc.vector.tensor_copy(
                                ys[:, half, nh * 512 : (nh + 1) * 512], psp[:]
                            )
                        if split_dma:
                            nc.sync.dma_start(
                                y_out[tt * P : (tt + 1) * P, :], ys[:, half, :]
                            )
                    if not split_dma:
                        t0 = (b * (T // P) + 2 * sq) * P
                        nc.sync.dma_start(
                            y_out[t0 : t0 + 2 * P, :].rearrange(
                                "(s p) d -> p s d", s=2
                            ),
                            ys[:],
                        )

                return [pv_half(0), pv_half(1), norm, tpev, proj]

            # ---- main schedule ----
            # Emission order interleaves batch 1's small blocks into late
            # batch 0 so the ACT (exp) feed never thins out at the batch
            # boundary; positions are indices into this order.
            blocks = [
                (0, 0), (0, 1), (0, 2), (0, 3), (0, 4), (0, 5),
                (1, 0), (0, 6), (1, 1), (0, 7),
                (1, 2), (1, 3), (1, 7), (1, 6), (1, 5), (1, 4),
            ]
            Gpre = []
            acc = 0
            for _, ss in blocks:
                Gpre.append(acc)
                acc += ss + 1
            groups_total = acc
            # tc j's pieces enqueue after position p, force-drain before the
            # first position whose block reads tc j's q/k
            tc_after = {0: 1, 1: 2, 2: 3, 3: 4, 5: 5, 7: 6, 9: 7}
            tc_deadline = {1: 2, 2: 4, 3: 6, 4: 6, 5: 10, 6: 12, 7: 12}

            # tc0: q/k inline so B00's scores (the first exp feed) emit
            # ASAP; v work rides the filler queue (needed by outputs(B00)
            # at position 2)
            tc0 = make_tc_pieces(0)
            for piece in tc0[0:6]:  # q + k chains inline (B00 needs them)
                piece()
            for piece in tc0[6:10]:  # v work rides the filler queue
                filler.append((2, piece))
            pend = {}
            for i, (b, sq) in enumerate(blocks):
                force_drain(i)
                pt = emit_scores(b, sq)
                pend[i] = (b, sq, pt)
                if i - 2 in pend:
                    bb, ss, pp = pend.pop(i - 2)
                    for u in make_output_units(bb, ss, pp):
                        filler.append((i + 2, u))
                if i in tc_after:
                    j = tc_after[i]
                    for piece in make_tc_pieces(j):
                        filler.append((tc_deadline[j], piece))
            # tail: interleave the last two blocks' stages so PE and DVE
            # overlap instead of serializing cross-engine round-trips. The
            # last block's PSUM tiles come from the (now idle) score banks
            # so the ps_sm single-slot ring doesn't force serialization,
            # and its y DMAs are split per 128-row tile.
            pop_fillers(len(filler))
            (b14, s14, p14), (b15, s15, p15) = pend[14], pend[15]
            u14 = make_output_units(b14, s14, p14, split_dma=True)
            u15 = make_output_units(
                b15, s15, p15, pool=ps_st, ptag="st", split_dma=True
            )
            for fn in (u14[0], u14[1], u14[2], u15[0], u15[1], u14[3],
                       u14[4], u15[2], u15[3], u15[4]):
                fn()

    nc.compile()
    return nc


def get_nc():
    global _CACHED_NC
    if _CACHED_NC is None:
        _CACHED_NC = build_nc()
    return _CACHED_NC


def make_in_maps(x, w_qkv, b_qkv, w_proj):
    bf = ml_dtypes.bfloat16
    e4 = ml_dtypes.float8_e4m3
    e5 = ml_dtypes.float8_e5m2
    x = np.asarray(x, dtype=np.float32).reshape(TA, D)
    w_qkv = np.asarray(w_qkv, dtype=np.float32)
    b_qkv = np.asarray(b_qkv, dtype=np.float32)
    w_proj = np.asarray(w_proj, dtype=np.float32)
    xT = np.ascontiguousarray(x.T)  # [D, TA] fp32, replicated
    x8 = xT.astype(e4)
    x8r = (xT - x8.astype(np.float32)).astype(e4)

    def wpack(ws):
        # 64x scale keeps the 0.02-std weights out of e4m3's subnormal
        # range; the evict rescales by 1/64
        ws = np.ascontiguousarray(ws) * 64.0
        w8 = ws.astype(e4)
        w8r = (ws - w8.astype(np.float32)).astype(e4)
        return np.ascontiguousarray(np.stack([w8, w8r], axis=1))  # [D, 2, P]
    # fp8e5 identity + strict-upper-triangular additive mask (-1536); the
    # same [128,128] triangle serves both diagonal chunks. Slot 1 of each
    # DoubleRow pair is zeros.
    i8 = np.zeros((P, 2, P), dtype=e5)
    i8[:, 0, :] = np.eye(P, dtype=np.float32).astype(e5)
    m8 = np.zeros((P, 2, P), dtype=e5)
    kk = np.arange(P)[:, None]
    qq = np.arange(P)[None, :]
    m8[:, 0, :] = np.where(kk > qq, -1536.0, 0.0).astype(e5)
    in_maps = []
    for c in range(N_CORES):
        lo = 2 * c * DH  # first feature column of this core's 2 heads
        in_maps.append(
            {
                "x8": x8,
                "x8r": x8r,
                "wq": wpack(w_qkv[:, lo : lo + P]),
                "wk": wpack(w_qkv[:, D + lo : D + lo + P]),
                "wv": wpack(w_qkv[:, 2 * D + lo : 2 * D + lo + P]),
                "bq": np.ascontiguousarray(b_qkv[lo : lo + P][:, None]),
                "bk": np.ascontiguousarray(b_qkv[D + lo : D + lo + P][:, None]),
                "wp": np.ascontiguousarray(w_proj[lo : lo + P, :]).astype(bf),
                "i8": i8,
                "m8": m8,
            }
        )
    return in_maps


def gather(results, b_qkv, w_proj, b_proj):
    b_qkv = np.asarray(b_qkv, dtype=np.float32)
    w_proj = np.asarray(w_proj, dtype=np.float32)
    b_proj = np.asarray(b_proj, dtype=np.float32)
    y = np.zeros((TA, D), dtype=np.float32)
    for c in range(N_CORES):
        y += np.asarray(results[c]["y"], dtype=np.float32)
    # exact host-side fold of the v-bias and projection bias:
    # softmax rows sum to 1, so the v-bias passes through attention intact.
    y += b_qkv[2 * D : 3 * D] @ w_proj + b_proj
    return y.reshape(B, T, D)


def run(x, w_qkv, b_qkv, w_proj, b_proj, trace=False, **spmd_kwargs):
    nc = get_nc()
    in_maps = make_in_maps(x, w_qkv, b_qkv, w_proj)
    res = run_bass_kernel_spmd(
        nc, in_maps, list(range(N_CORES)), trace=trace, **spmd_kwargs
    )
    return gather(res.results, b_qkv, w_proj, b_proj), res


def kernel(x, w_qkv, b_qkv, w_proj, b_proj):
    y, _ = run(x, w_qkv, b_qkv, w_proj, b_proj)
    return y


# revision 55
# speedup vs baseline: 1.3139x; 1.0283x over previous
"""Multi-head causal self-attention (B=2, T=2048, D=1024, H=16, Dh=64) on 8
Trainium2 NeuronCores.

Sharding (Megatron-style tensor parallel over heads):
  - Each core owns 2 heads (core c -> heads 2c, 2c+1) for both batch rows.
  - w_qkv column-sharded per core ([1024, 128] per q/k/v, bf16 on host).
  - w_proj row-sharded ([128, 1024] bf16); cores emit partial projection
    outputs which the host sums (plus bias terms folded exactly on host).
  - x replicated, passed pre-transposed AND pre-cast: xT [1024, 4096] bf16
    (halves the input DMA vs fp32 and removes the on-device cast).

Device-side per core:
  qT/kT/vT = W^T x^T on PE. q evicted as fp8e4 (q8); k as an fp8e4 pair
  (k8, k8r) slot-interleaved in one tile, k8r = (k + bk) - k8 being the
  quantization residual. Scores are fp8 DoubleRow matmuls (0.5 cyc/col):
  stationary tiles (k8, k8r), moving tiles (q8, q8) via a stride-0
  broadcast AP -> (k8+k8r)^T q8: K accurate to ~13 bits, Q to e4m3, logit
  error ~2.5% of a 0.41-std logit => ~1.2e-2 final rel err, inside 2e-2.
  Causal masks for the two diagonal chunks fold in as fp8e5
  identity.T @ (-1536 triangle) DoubleRow accumulates; the odd-diagonal
  chunk computes only its valid 128-query half and PV skips it for the
  first query sub-block. exp on ACT per 2-chunk PSUM bank; PV in bf16
  with a ones-column in V2 producing softmax denominators in col 0;
  normalize on DVE; 4 PE transposes packed per PSUM bank (tile_position
  places head 1 at partitions 64..127) + one DVE evict; projection per
  256-row block with a single batched y DMA.

Schedule: ACT (exp) is the second-busiest engine (~88us) after PE
(~93us) and exp backlog is capped by 3 score PSUM banks (~1.8us), so any
contiguous >2us stretch of non-score PE work starves ACT. The emitter
therefore weaves: score groups are emitted back-to-back while qkv chains
(split into ~0.9us pieces) and block output work (PV/normalize/proj,
pipelined >=2 blocks behind) are popped from a filler queue between
groups at a rate proportional to the remaining filler/group ratio.
Forced drains keep feasibility: tc j before the blocks that read it, and
output units before their pt pool slots are reused (4-block window).
"""

import numpy as np
import ml_dtypes

import concourse.bacc as bacc
import concourse.bass as bass
import concourse.mybir as mybir
import concourse.tile as tile
from concourse.bass_utils import run_bass_kernel_spmd
from concourse.masks import make_identity

N_CORES = 8
B = 2
T = 2048
D = 1024
H = 16
DH = 64
TA = B * T  # 4096 rows total
P = 128
NQB = T // P  # 16 key chunks per batch
KC = D // P  # 8 contraction chunks for qkv
SQ = 256  # superblock query count
BF = mybir.dt.bfloat16
F32 = mybir.dt.float32
F8 = mybir.dt.float8e4
F8E5 = mybir.dt.float8e5
DR = mybir.MatmulPerfMode.DoubleRow
ADD = mybir.AluOpType.add
SUB = mybir.AluOpType.subtract
MUL = mybir.AluOpType.mult

_CACHED_NC = None


def build_nc():
    """Build the per-core Bass program (identical on all 8 cores)."""
    nc = bacc.Bacc("TRN2", target_bir_lowering=False, debug=False, num_devices=N_CORES)

    NS = TA // 512  # x DMA chunks (one per qkv T-chunk)
    x8_in = nc.dram_tensor("x8", [P, NS, KC, 512], F8, kind="ExternalInput").ap()
    x8r_in = nc.dram_tensor("x8r", [P, NS, KC, 512], F8, kind="ExternalInput").ap()
    wq_in = nc.dram_tensor("wq", [P, KC, 2, P], F8, kind="ExternalInput").ap()
    wk_in = nc.dram_tensor("wk", [P, KC, 2, P], F8, kind="ExternalInput").ap()
    wv_in = nc.dram_tensor("wv", [P, KC, 2, P], F8, kind="ExternalInput").ap()
    bq_in = nc.dram_tensor("bq", [P, 1], F32, kind="ExternalInput").ap()
    bk_in = nc.dram_tensor("bk", [P, 1], F32, kind="ExternalInput").ap()
    wp_in = nc.dram_tensor("wp", [P, D], BF, kind="ExternalInput").ap()
    i8_in = nc.dram_tensor("i8", [P, 2, P], F8E5, kind="ExternalInput").ap()
    m8_in = nc.dram_tensor("m8", [P, 2, P], F8E5, kind="ExternalInput").ap()
    y_out = nc.dram_tensor("y", [TA, D], BF, kind="ExternalOutput").ap()

    with tile.TileContext(nc) as tc:
        with (
            tc.tile_pool(name="const", bufs=1) as const,
            tc.tile_pool(name="xts", bufs=1) as xts,
            tc.tile_pool(name="qkv", bufs=1) as qkv,
            tc.tile_pool(name="ptp", bufs=8) as ptp,
            tc.tile_pool(name="osml", bufs=8) as osml,
            tc.tile_pool(name="rcp", bufs=8) as rcp,
            tc.tile_pool(name="ystage", bufs=3) as ystage,
            tc.tile_pool(name="ps_st", bufs=3, space="PSUM") as ps_st,
            tc.tile_pool(name="ps_qk", bufs=2, space="PSUM") as ps_qk,
            tc.tile_pool(name="ps_pj", bufs=2, space="PSUM") as ps_pj,
            tc.tile_pool(name="ps_sm", bufs=1, space="PSUM") as ps_sm,
        ):
            # ---- constants ----
            ident = const.tile([P, P], BF)
            make_identity(nc, ident[:])
            bq_sb = const.tile([P, 1], F32)
            nc.sync.dma_start(bq_sb[:], bq_in[:])
            bk_sb = const.tile([P, 1], F32)
            nc.sync.dma_start(bk_sb[:], bk_in[:])
            # touch Exp now so the ACT table load (1.3us) is off the
            # critical path of the first real exp
            warm_act = const.tile([P, 1], BF)
            nc.scalar.activation(
                warm_act[:], bq_sb[:], mybir.ActivationFunctionType.Exp
            )

            # ---- xT load (bf16 from host): one strided DMA per 512-col
            # T-chunk ([128, 8, 512] = all contraction chunks at once) so
            # tc j is runnable after j+1 DMAs; weight DMAs woven between
            # the first x chunks in need-order ----
            x8_sb = xts.tile([P, NS, KC, 512], F8)
            x8r_sb = xts.tile([P, NS, KC, 512], F8)
            i8_sb = const.tile([P, 2, P], F8E5)
            m8_sb = const.tile([P, 2, P], F8E5)
            wp_sb = const.tile([P, D], BF)
            w_sb = {}
            for name in ("q", "k", "v"):
                w_sb[name] = const.tile([P, KC, 2, P], F8, name=f"w{name}sb")
            # startup-critical DMAs in exact need-order of the first qkv
            # chains and B00's scores (masks before the first diag group);
            # all host-laid-out contiguous per partition
            nc.sync.dma_start(w_sb["q"][:], wq_in[:])
            nc.sync.dma_start(x8_sb[:, 0], x8_in[:, 0])
            nc.sync.dma_start(w_sb["k"][:], wk_in[:])
            nc.sync.dma_start(x8r_sb[:, 0], x8r_in[:, 0])
            nc.sync.dma_start(w_sb["v"][:], wv_in[:])
            nc.sync.dma_start(i8_sb[:], i8_in[:])
            nc.sync.dma_start(m8_sb[:], m8_in[:])
            for s in range(1, NS):
                nc.sync.dma_start(x8_sb[:, s], x8_in[:, s])
                nc.sync.dma_start(x8r_sb[:, s], x8r_in[:, s])
                if s == 1:  # needed from position 2 on; off the hot queue
                    nc.scalar.dma_start(wp_sb[:], wp_in[:])

            # ---- PE warmup: dependency-free matmuls cover the x-load ramp
            # so HAM reaches 2.4 GHz before the first real qkv matmul ----
            wm = ps_pj.tile([P, 512], F32, name="warm", tag="pj")

            def warm(n):
                for _ in range(n):
                    nc.tensor.matmul(
                        wm[:, 0:P], ident[:], ident[:], start=True, stop=True
                    )

            warm(38)

            # ---- persistent activation tiles ----
            q8_sb = qkv.tile([P, B, T], F8)
            k2_sb = qkv.tile([P, 2, B, T], F8)
            vT_sb = qkv.tile([P, B, T], BF)
            # V2 per (b, key-chunk): [1 | V_h0 (64) | 1 | V_h1 (64)]
            V2 = qkv.tile([P, B, NQB, 130], BF)
            nc.vector.memset(V2[:, :, :, 0], 1.0)
            nc.vector.memset(V2[:, :, :, 65], 1.0)
            attn_oT = qkv.tile([P, TA], BF)

            # ---- qkv T-chunk as 10 filler pieces (~0.45us PE each) ----
            # q/k/v in fp8 DoubleRow with full residual compensation:
            # psum = (w8+w8r)^T x8 + w8^T x8r  (w pre-scaled 64x on host,
            # rescaled at evict; only the negligible w8r*x8r term is
            # dropped). 3072 PE cycles per [128,512] tile vs 4096 bf16.
            SW64 = 1.0 / 64.0

            def make_tc_pieces(tcg):
                b = tcg // 4
                col = (tcg % 4) * 512
                held = {}

                def chain_a(blk, half):
                    def f():
                        if half == 0:
                            held[blk] = ps_qk.tile(
                                [P, 512], F32, name="pqk", tag="qk"
                            )
                        pst = held[blk]
                        for c in range(half * 4, half * 4 + 4):
                            x8b = (
                                x8_sb[:, tcg, c, :]
                                .unsqueeze(1)
                                .broadcast_to([P, 2, 512])
                            )
                            nc.tensor.matmul(
                                pst[:],
                                w_sb[blk][:, c, :, :],
                                x8b,
                                start=(c == 0),
                                stop=False,
                                perf_mode=DR,
                            )

                    return f

                def chain_b(blk):
                    def f():
                        pst = held[blk]
                        for cp in range(0, KC, 2):
                            nc.tensor.matmul(
                                pst[:],
                                w_sb[blk][:, cp : cp + 2, 0, :],
                                x8r_sb[:, tcg, cp : cp + 2, :],
                                start=False,
                                stop=(cp == KC - 2),
                                perf_mode=DR,
                            )
                        if blk == "q":
                            nc.vector.tensor_scalar(
                                q8_sb[:, b, col : col + 512],
                                pst[:],
                                SW64,
                                bq_sb[:],
                                op0=MUL,
                                op1=ADD,
                            )
                        elif blk == "k":
                            k8 = k2_sb[:, 0, b, col : col + 512]
                            nc.vector.tensor_scalar(
                                k8, pst[:], SW64, bk_sb[:], op0=MUL, op1=ADD
                            )
                            # residual drops bk's own quantization residual
                            # (exact when bk == 0)
                            nc.vector.scalar_tensor_tensor(
                                k2_sb[:, 1, b, col : col + 512],
                                pst[:],
                                SW64,
                                k8,
                                op0=MUL,
                                op1=SUB,
                            )
                        else:
                            nc.vector.tensor_scalar(
                                vT_sb[:, b, col : col + 512],
                                pst[:],
                                SW64,
                                None,
                                op0=MUL,
                            )

                    return f

                def vfix():
                    bs = (tcg % 4) * 4
                    vtp = ps_qk.tile([P, 4, P], BF, name="vtp", tag="qk")
                    for j in range(4):
                        s = bs + j
                        nc.tensor.matmul(
                            vtp[:, j, :],
                            vT_sb[:, b, s * P : (s + 1) * P],
                            ident[:],
                            is_transpose=True,
                            start=(j == 0),
                            stop=(j == 3),
                        )
                    nc.vector.tensor_copy(
                        V2[:, b, bs : bs + 4, :].rearrange(
                            "p s (h x) -> p s h x", h=2
                        )[:, :, :, 1:65],
                        vtp[:].rearrange("p s (h d) -> p s h d", h=2),
                    )

                return [
                    chain_a("q", 0),
                    chain_a("q", 1),
                    chain_b("q"),
                    chain_a("k", 0),
                    chain_a("k", 1),
                    chain_b("k"),
                    chain_a("v", 0),
                    chain_a("v", 1),
                    chain_b("v"),
                    vfix,
                ]

            # ---- filler queue: (deadline_position, closure) FIFO ----
            filler = []
            groups_done = [0]

            def pop_fillers(k):
                for _ in range(min(k, len(filler))):
                    filler.pop(0)[1]()

            def force_drain(upto):
                due = [u for u in filler if u[0] <= upto]
                filler[:] = [u for u in filler if u[0] > upto]
                for _, fn in due:
                    fn()

            def weave():
                groups_done[0] += 1
                if not filler:
                    return
                left = max(groups_total - groups_done[0], 1)
                k = -(-len(filler) // left)  # global even spread
                # urgency: items due at the earliest deadline must drain
                # evenly over the groups remaining before that position
                d0 = min(f[0] for f in filler)
                n_due = sum(1 for f in filler if f[0] <= d0)
                gb = (Gpre[d0] if d0 < len(Gpre) else groups_total) - groups_done[0]
                k = max(k, n_due if gb <= 0 else -(-n_due // gb))
                pop_fillers(min(k, 8))

            # ---- scores for one 256-query superblock (fp8 DoubleRow) ----
            def emit_scores(b, sq):
                nk = 2 * sq + 2
                q0 = b * T + sq * SQ
                pt = {}
                for h in (0, 1):
                    pt[h] = ptp.tile([P, NQB, SQ], BF, name="ptt", tag="pt")
                for g in range(0, nk, 2):
                    diag = g == nk - 2
                    st = {}
                    for h in (0, 1):
                        st[h] = ps_st.tile([P, 512], F32, name="st", tag="st")
                    for j in (0, 1):
                        c = g + j
                        odd_diag = diag and j == 1
                        for h in (0, 1):
                            hp = h * DH
                            lhsT = k2_sb[hp : hp + DH, :, b, c * P : (c + 1) * P]
                            if odd_diag:
                                # valid queries (second 128) land at bank
                                # cols [256:384] so exp covers [0:384]
                                # contiguously; PV reads this chunk's qh=1
                                # data at pt[.., nk-1, 0:128]
                                q8b = (
                                    q8_sb[hp : hp + DH, b, sq * SQ + P : sq * SQ + SQ]
                                    .unsqueeze(1)
                                    .broadcast_to([DH, 2, P])
                                )
                                dst = st[h][:, SQ : SQ + P]
                            else:
                                q8b = (
                                    q8_sb[hp : hp + DH, b, sq * SQ : sq * SQ + SQ]
                                    .unsqueeze(1)
                                    .broadcast_to([DH, 2, SQ])
                                )
                                dst = st[h][:, j * SQ : (j + 1) * SQ]
                            nc.tensor.matmul(
                                dst,
                                lhsT,
                                q8b,
                                start=(j == 0),
                                stop=(j == 1) and not diag,
                                perf_mode=DR,
                            )
                    if diag:
                        # triangular -1536 mask: even chunk's first 128
                        # queries, odd chunk's (valid) last 128 queries
                        for h in (0, 1):
                            nc.tensor.matmul(
                                st[h][:, 0:P],
                                i8_sb[:],
                                m8_sb[:],
                                start=False,
                                stop=False,
                                perf_mode=DR,
                            )
                            nc.tensor.matmul(
                                st[h][:, SQ : SQ + P],
                                i8_sb[:],
                                m8_sb[:],
                                start=False,
                                stop=True,
                                perf_mode=DR,
                            )
                    ecols = SQ + P if diag else 2 * SQ
                    for h in (0, 1):
                        nc.scalar.activation(
                            pt[h][:, g : g + 2, :].rearrange("p a b -> p (a b)")[
                                :, 0:ecols
                            ],
                            st[h][:, 0:ecols],
                            mybir.ActivationFunctionType.Exp,
                            scale=0.125,
                        )
                    weave()
                return pt

            # ---- output work for one superblock: 4 filler units ----
            def make_output_units(
                b, sq, pt, pool=None, ptag=None, split_dma=False, use_act=False
            ):
                """use_act: offload normalize + half the proj evicts to the
                ACT engine — only valid after the last exp (tail blocks)."""
                nk = 2 * sq + 2
                held = {}
                pool = pool or ps_sm
                ptag = ptag or "sm"

                def pv_half(hs):
                    def f():
                        if hs == 0:
                            held["pvt"] = pool.tile(
                                [P, 4 * 65], F32, name="pvt", tag=ptag
                            )
                        pvt = held["pvt"]
                        for qh in (0, 1):
                            off = (2 * hs + qh) * 65
                            nck = nk - 1 if qh == 0 else nk
                            for c in range(nck):
                                # odd-diag chunk's valid (qh=1) data is
                                # stored at cols 0:128 (exp trim)
                                qoff = 0 if (qh == 1 and c == nk - 1) else qh * P
                                nc.tensor.matmul(
                                    pvt[:, off : off + 65],
                                    pt[hs][:, c, qoff : qoff + P],
                                    V2[:, b, c, hs * 65 : hs * 65 + 65],
                                    start=(hs == 0 and qh == 0 and c == 0),
                                    stop=(c == nck - 1),
                                )

                    return f

                def norm():
                    pvt = held["pvt"]
                    osbs = []
                    for h in (0, 1):
                        for qh in (0, 1):
                            off = (2 * h + qh) * 65
                            r = rcp.tile([P, 1], F32, name="rr", tag="rr")
                            nc.vector.reciprocal(r[:], pvt[:, off : off + 1])
                            osb = osml.tile([P, DH], BF, name="osb")
                            if use_act:
                                nc.scalar.activation(
                                    osb[:],
                                    pvt[:, off + 1 : off + 65],
                                    mybir.ActivationFunctionType.Copy,
                                    scale=r[:],
                                )
                            else:
                                nc.vector.tensor_scalar_mul(
                                    osb[:], pvt[:, off + 1 : off + 65], r[:]
                                )
                            osbs.append((h, qh, osb))
                    held["osbs"] = osbs

                def tpev():
                    tp = pool.tile([P, SQ], BF, name="tp", tag=ptag)
                    for h, qh, osb in held["osbs"]:
                        hp = h * DH
                        nc.tensor.matmul(
                            tp[hp : hp + DH, qh * P : (qh + 1) * P],
                            osb[:],
                            ident[:],
                            is_transpose=True,
                            tile_position=(0, hp),
                            start=(qh == 0),
                            stop=(qh == 1),
                        )
                    nc.vector.tensor_copy(
                        attn_oT[:, b * T + sq * SQ : b * T + (sq + 1) * SQ], tp[:]
                    )

                def proj():
                    ys = ystage.tile([P, 2, D], BF, name="ys")
                    for half in range(2):
                        tt = b * (T // P) + 2 * sq + half
                        for nh in range(2):
                            psp = ps_pj.tile([P, 512], F32, name="psp", tag="pj")
                            nc.tensor.matmul(
                                psp[:],
                                attn_oT[:, tt * P : (tt + 1) * P],
                                wp_sb[:, nh * 512 : (nh + 1) * 512],
                                start=True,
                                stop=True,
                            )
                            if use_act and nh == 1:
                                nc.scalar.activation(
                                    ys[:, half, nh * 512 : (nh + 1) * 512],
                                    psp[:],
                                    mybir.ActivationFunctionType.Copy,
                                )
                            else:
                                nc.vector.tensor_copy(
                                    ys[:, half, nh * 512 : (nh + 1) * 512], psp[:]
                                )
                        if split_dma:
                            nc.sync.dma_start(
                                y_out[tt * P : (tt + 1) * P, :], ys[:, half, :]
                            )
                    if not split_dma:
                        t0 = (b * (T // P) + 2 * sq) * P
                        nc.sync.dma_start(
                            y_out[t0 : t0 + 2 * P, :].rearrange(
                                "(s p) d -> p s d", s=2
                            ),
                            ys[:],
                        )

                return [pv_half(0), pv_half(1), norm, tpev, proj]

            # ---- main schedule ----
            # Emission order interleaves batch 1's small blocks into late
            # batch 0 so the ACT (exp) feed never thins out at the batch
            # boundary; positions are indices into this order.
            blocks = [
                (0, 0), (0, 1), (0, 2), (0, 3), (0, 4), (0, 5),
                (1, 0), (0, 6), (1, 1), (0, 7),
                (1, 2), (1, 3), (1, 7), (1, 6), (1, 5), (1, 4),
            ]
            Gpre = []
            acc = 0
            for _, ss in blocks:
                Gpre.append(acc)
                acc += ss + 1
            groups_total = acc
            # tc j's pieces enqueue after position p, force-drain before the
            # first position whose block reads tc j's q/k
            tc_after = {0: 1, 1: 2, 2: 3, 3: 4, 5: 5, 7: 6, 9: 7}
            tc_deadline = {1: 2, 2: 4, 3: 6, 4: 6, 5: 10, 6: 12, 7: 12}

            # tc0: q/k inline so B00's scores (the first exp feed) emit
            # ASAP; v work rides the filler queue (needed by outputs(B00)
            # at position 2)
            tc0 = make_tc_pieces(0)
            # q + k chains inline (B00 needs them), ordered to match DMA
            # arrival (x8s0, w_k, x8r s0) with warm matmuls plugging the
            # remaining DMA-latency holes so the PE clock never resets
            for piece, nw in zip(
                (tc0[0], tc0[1], tc0[3], tc0[4], tc0[2], tc0[5]),
                (0, 0, 8, 0, 8, 0),
            ):
                if nw:
                    warm(nw)
                piece()
            for piece in tc0[6:10]:  # v work rides the filler queue
                filler.append((2, piece))
            pend = {}
            for i, (b, sq) in enumerate(blocks):
                force_drain(i)
                pt = emit_scores(b, sq)
                pend[i] = (b, sq, pt)
                if i - 2 in pend:
                    bb, ss, pp = pend.pop(i - 2)
                    for u in make_output_units(bb, ss, pp):
                        filler.append((i + 2, u))
                if i in tc_after:
                    j = tc_after[i]
                    for piece in make_tc_pieces(j):
                        filler.append((tc_deadline[j], piece))
            # tail: interleave the last two blocks' stages so PE and DVE
            # overlap instead of serializing cross-engine round-trips. The
            # last block's PSUM tiles come from the (now idle) score banks
            # so the ps_sm single-slot ring doesn't force serialization,
            # and its y DMAs are split per 128-row tile.
            pop_fillers(len(filler))
            (b14, s14, p14), (b15, s15, p15) = pend[14], pend[15]
            u14 = make_output_units(b14, s14, p14, split_dma=True, use_act=True)
            u15 = make_output_units(
                b15, s15, p15, pool=ps_st, ptag="st", split_dma=True,
                use_act=True,
            )
            for fn in (u14[0], u14[1], u14[2], u15[0], u15[1], u14[3],
                       u14[4], u15[2], u15[3], u15[4]):
                fn()

    nc.compile()
    return nc


def get_nc():
    global _CACHED_NC
    if _CACHED_NC is None:
        _CACHED_NC = build_nc()
    return _CACHED_NC


def make_in_maps(x, w_qkv, b_qkv, w_proj):
    bf = ml_dtypes.bfloat16
    e4 = ml_dtypes.float8_e4m3
    e5 = ml_dtypes.float8_e5m2
    x = np.asarray(x, dtype=np.float32).reshape(TA, D)
    w_qkv = np.asarray(w_qkv, dtype=np.float32)
    b_qkv = np.asarray(b_qkv, dtype=np.float32)
    w_proj = np.asarray(w_proj, dtype=np.float32)
    xT = np.ascontiguousarray(x.T)  # [D, TA] fp32, replicated

    def xlay(a):
        # device layout [P, NS, KC, 512]: chunk s contiguous per partition
        return np.ascontiguousarray(
            a.reshape(KC, P, TA // 512, 512).transpose(1, 2, 0, 3)
        )

    x8 = xT.astype(e4)
    x8r = xlay((xT - x8.astype(np.float32)).astype(e4))
    x8 = xlay(x8)

    def wpack(ws):
        # 64x scale keeps the 0.02-std weights out of e4m3's subnormal
        # range; the evict rescales by 1/64. Layout [P, KC, 2, P].
        ws = np.ascontiguousarray(ws) * 64.0
        w8 = ws.astype(e4)
        w8r = (ws - w8.astype(np.float32)).astype(e4)
        wp2 = np.stack([w8, w8r], axis=0)  # [2, D, P]
        return np.ascontiguousarray(
            wp2.reshape(2, KC, P, P).transpose(2, 1, 0, 3)
        )
    # fp8e5 identity + strict-upper-triangular additive mask (-1536); the
    # same [128,128] triangle serves both diagonal chunks. Slot 1 of each
    # DoubleRow pair is zeros.
    i8 = np.zeros((P, 2, P), dtype=e5)
    i8[:, 0, :] = np.eye(P, dtype=np.float32).astype(e5)
    m8 = np.zeros((P, 2, P), dtype=e5)
    kk = np.arange(P)[:, None]
    qq = np.arange(P)[None, :]
    m8[:, 0, :] = np.where(kk > qq, -1536.0, 0.0).astype(e5)
    in_maps = []
    for c in range(N_CORES):
        lo = 2 * c * DH  # first feature column of this core's 2 heads
        in_maps.append(
            {
                "x8": x8,
                "x8r": x8r,
                "wq": wpack(w_qkv[:, lo : lo + P]),
                "wk": wpack(w_qkv[:, D + lo : D + lo + P]),
                "wv": wpack(w_qkv[:, 2 * D + lo : 2 * D + lo + P]),
                "bq": np.ascontiguousarray(b_qkv[lo : lo + P][:, None]),
                "bk": np.ascontiguousarray(b_qkv[D + lo : D + lo + P][:, None]),
                "wp": np.ascontiguousarray(w_proj[lo : lo + P, :]).astype(bf),
                "i8": i8,
                "m8": m8,
            }
        )
    return in_maps


def gather(results, b_qkv, w_proj, b_proj):
    b_qkv = np.asarray(b_qkv, dtype=np.float32)
    w_proj = np.asarray(w_proj, dtype=np.float32)
    b_proj = np.asarray(b_proj, dtype=np.float32)
    y = np.zeros((TA, D), dtype=np.float32)
    for c in range(N_CORES):
        y += np.asarray(results[c]["y"], dtype=np.float32)
    # exact host-side fold of the v-bias and projection bias:
    # softmax rows sum to 1, so the v-bias passes through attention intact.
    y += b_qkv[2 * D : 3 * D] @ w_proj + b_proj
    return y.reshape(B, T, D)


def run(x, w_qkv, b_qkv, w_proj, b_proj, trace=False, **spmd_kwargs):
    nc = get_nc()
    in_maps = make_in_maps(x, w_qkv, b_qkv, w_proj)
    res = run_bass_kernel_spmd(
        nc, in_maps, list(range(N_CORES)), trace=trace, **spmd_kwargs
    )
    return gather(res.results, b_qkv, w_proj, b_proj), res


def kernel(x, w_qkv, b_qkv, w_proj, b_proj):
    y, _ = run(x, w_qkv, b_qkv, w_proj, b_proj)
    return y


# revision 62
# speedup vs baseline: 1.3247x; 1.0082x over previous
"""Multi-head causal self-attention (B=2, T=2048, D=1024, H=16, Dh=64) on 8
Trainium2 NeuronCores.

Sharding (Megatron-style tensor parallel over heads):
  - Each core owns 2 heads (core c -> heads 2c, 2c+1) for both batch rows.
  - w_qkv column-sharded per core ([1024, 128] per q/k/v, bf16 on host).
  - w_proj row-sharded ([128, 1024] bf16); cores emit partial projection
    outputs which the host sums (plus bias terms folded exactly on host).
  - x replicated, passed pre-transposed AND pre-cast: xT [1024, 4096] bf16
    (halves the input DMA vs fp32 and removes the on-device cast).

Device-side per core:
  qT/kT/vT = W^T x^T on PE. q evicted as fp8e4 (q8); k as an fp8e4 pair
  (k8, k8r) slot-interleaved in one tile, k8r = (k + bk) - k8 being the
  quantization residual. Scores are fp8 DoubleRow matmuls (0.5 cyc/col):
  stationary tiles (k8, k8r), moving tiles (q8, q8) via a stride-0
  broadcast AP -> (k8+k8r)^T q8: K accurate to ~13 bits, Q to e4m3, logit
  error ~2.5% of a 0.41-std logit => ~1.2e-2 final rel err, inside 2e-2.
  Causal masks for the two diagonal chunks fold in as fp8e5
  identity.T @ (-1536 triangle) DoubleRow accumulates; the odd-diagonal
  chunk computes only its valid 128-query half and PV skips it for the
  first query sub-block. exp on ACT per 2-chunk PSUM bank; PV in bf16
  with a ones-column in V2 producing softmax denominators in col 0;
  normalize on DVE; 4 PE transposes packed per PSUM bank (tile_position
  places head 1 at partitions 64..127) + one DVE evict; projection per
  256-row block with a single batched y DMA.

Schedule: ACT (exp) is the second-busiest engine (~88us) after PE
(~93us) and exp backlog is capped by 3 score PSUM banks (~1.8us), so any
contiguous >2us stretch of non-score PE work starves ACT. The emitter
therefore weaves: score groups are emitted back-to-back while qkv chains
(split into ~0.9us pieces) and block output work (PV/normalize/proj,
pipelined >=2 blocks behind) are popped from a filler queue between
groups at a rate proportional to the remaining filler/group ratio.
Forced drains keep feasibility: tc j before the blocks that read it, and
output units before their pt pool slots are reused (4-block window).
"""

import numpy as np
import ml_dtypes

import concourse.bacc as bacc
import concourse.bass as bass
import concourse.mybir as mybir
import concourse.tile as tile
from concourse.bass_utils import run_bass_kernel_spmd
from concourse.masks import make_identity

N_CORES = 8
B = 2
T = 2048
D = 1024
H = 16
DH = 64
TA = B * T  # 4096 rows total
P = 128
NQB = T // P  # 16 key chunks per batch
KC = D // P  # 8 contraction chunks for qkv
SQ = 256  # superblock query count
BF = mybir.dt.bfloat16
F32 = mybir.dt.float32
F8 = mybir.dt.float8e4
F8E5 = mybir.dt.float8e5
DR = mybir.MatmulPerfMode.DoubleRow
ADD = mybir.AluOpType.add
SUB = mybir.AluOpType.subtract
MUL = mybir.AluOpType.mult

_CACHED_NC = None


def build_nc():
    """Build the per-core Bass program (identical on all 8 cores)."""
    nc = bacc.Bacc("TRN2", target_bir_lowering=False, debug=False, num_devices=N_CORES)

    NS = TA // 512  # x DMA chunks (one per qkv T-chunk)
    x8_in = nc.dram_tensor("x8", [P, NS, KC, 512], F8, kind="ExternalInput").ap()
    x8r_in = nc.dram_tensor("x8r", [P, NS, KC, 512], F8, kind="ExternalInput").ap()
    wq_in = nc.dram_tensor("wq", [P, KC, 2, P], F8, kind="ExternalInput").ap()
    wk_in = nc.dram_tensor("wk", [P, KC, 2, P], F8, kind="ExternalInput").ap()
    wv_in = nc.dram_tensor("wv", [P, KC, 2, P], F8, kind="ExternalInput").ap()
    bq_in = nc.dram_tensor("bq", [P, 1], F32, kind="ExternalInput").ap()
    bk_in = nc.dram_tensor("bk", [P, 1], F32, kind="ExternalInput").ap()
    wp_in = nc.dram_tensor("wp", [P, D], BF, kind="ExternalInput").ap()
    i8_in = nc.dram_tensor("i8", [P, 2, P], F8E5, kind="ExternalInput").ap()
    m8_in = nc.dram_tensor("m8", [P, 2, P], F8E5, kind="ExternalInput").ap()
    y_out = nc.dram_tensor("y", [TA, D], BF, kind="ExternalOutput").ap()

    with tile.TileContext(nc) as tc:
        with (
            tc.tile_pool(name="const", bufs=1) as const,
            tc.tile_pool(name="xts", bufs=1) as xts,
            tc.tile_pool(name="qkv", bufs=1) as qkv,
            tc.tile_pool(name="ptp", bufs=8) as ptp,
            tc.tile_pool(name="osml", bufs=8) as osml,
            tc.tile_pool(name="rcp", bufs=8) as rcp,
            tc.tile_pool(name="ystage", bufs=3) as ystage,
            tc.tile_pool(name="ps_st", bufs=3, space="PSUM") as ps_st,
            tc.tile_pool(name="ps_qk", bufs=2, space="PSUM") as ps_qk,
            tc.tile_pool(name="ps_pj", bufs=2, space="PSUM") as ps_pj,
            tc.tile_pool(name="ps_sm", bufs=1, space="PSUM") as ps_sm,
        ):
            # ---- constants ----
            ident = const.tile([P, P], BF)
            make_identity(nc, ident[:])
            bq_sb = const.tile([P, 1], F32)
            nc.sync.dma_start(bq_sb[:], bq_in[:])
            bk_sb = const.tile([P, 1], F32)
            nc.sync.dma_start(bk_sb[:], bk_in[:])
            # touch Exp now so the ACT table load (1.3us) is off the
            # critical path of the first real exp
            warm_act = const.tile([P, 1], BF)
            nc.scalar.activation(
                warm_act[:], bq_sb[:], mybir.ActivationFunctionType.Exp
            )

            # ---- xT load (bf16 from host): one strided DMA per 512-col
            # T-chunk ([128, 8, 512] = all contraction chunks at once) so
            # tc j is runnable after j+1 DMAs; weight DMAs woven between
            # the first x chunks in need-order ----
            x8_sb = xts.tile([P, NS, KC, 512], F8)
            x8r_sb = xts.tile([P, NS, KC, 512], F8)
            i8_sb = const.tile([P, 2, P], F8E5)
            m8_sb = const.tile([P, 2, P], F8E5)
            wp_sb = const.tile([P, D], BF)
            w_sb = {}
            for name in ("q", "k", "v"):
                w_sb[name] = const.tile([P, KC, 2, P], F8, name=f"w{name}sb")
            # startup-critical DMAs in exact need-order of the first qkv
            # chains and B00's scores (masks before the first diag group);
            # all host-laid-out contiguous per partition
            nc.sync.dma_start(w_sb["q"][:], wq_in[:])
            nc.sync.dma_start(x8_sb[:, 0], x8_in[:, 0])
            nc.sync.dma_start(w_sb["k"][:], wk_in[:])
            nc.sync.dma_start(x8r_sb[:, 0], x8r_in[:, 0])
            nc.sync.dma_start(w_sb["v"][:], wv_in[:])
            nc.sync.dma_start(i8_sb[:], i8_in[:])
            nc.sync.dma_start(m8_sb[:], m8_in[:])
            for s in range(1, NS):
                nc.sync.dma_start(x8_sb[:, s], x8_in[:, s])
                nc.sync.dma_start(x8r_sb[:, s], x8r_in[:, s])
                if s == 1:  # needed from position 2 on; off the hot queue
                    nc.scalar.dma_start(wp_sb[:], wp_in[:])

            # ---- PE warmup: dependency-free matmuls cover the x-load ramp
            # so HAM reaches 2.4 GHz before the first real qkv matmul ----
            wm = ps_pj.tile([P, 512], F32, name="warm", tag="pj")

            def warm(n):
                for _ in range(n):
                    nc.tensor.matmul(
                        wm[:, 0:P], ident[:], ident[:], start=True, stop=True
                    )

            warm(38)

            # ---- persistent activation tiles ----
            q8_sb = qkv.tile([P, B, T], F8)
            k2_sb = qkv.tile([P, 2, B, T], F8)
            vT_sb = qkv.tile([P, B, T], BF)
            # V2 per (b, key-chunk): [1 | V_h0 (64) | 1 | V_h1 (64)]
            V2 = qkv.tile([P, B, NQB, 130], BF)
            nc.vector.memset(V2[:, :, :, 0], 1.0)
            nc.vector.memset(V2[:, :, :, 65], 1.0)
            attn_oT = qkv.tile([P, TA], BF)

            # ---- qkv T-chunk as 10 filler pieces (~0.45us PE each) ----
            # q/k/v in fp8 DoubleRow with full residual compensation:
            # psum = (w8+w8r)^T x8 + w8^T x8r  (w pre-scaled 64x on host,
            # rescaled at evict; only the negligible w8r*x8r term is
            # dropped). 3072 PE cycles per [128,512] tile vs 4096 bf16.
            SW64 = 1.0 / 64.0

            def make_tc_pieces(tcg):
                b = tcg // 4
                col = (tcg % 4) * 512
                held = {}

                def chain_a(blk, half):
                    def f():
                        if half == 0:
                            held[blk] = ps_qk.tile(
                                [P, 512], F32, name="pqk", tag="qk"
                            )
                        pst = held[blk]
                        for c in range(half * 4, half * 4 + 4):
                            x8b = (
                                x8_sb[:, tcg, c, :]
                                .unsqueeze(1)
                                .broadcast_to([P, 2, 512])
                            )
                            nc.tensor.matmul(
                                pst[:],
                                w_sb[blk][:, c, :, :],
                                x8b,
                                start=(c == 0),
                                stop=False,
                                perf_mode=DR,
                            )

                    return f

                def chain_b(blk):
                    def f():
                        pst = held[blk]
                        for cp in range(0, KC, 2):
                            nc.tensor.matmul(
                                pst[:],
                                w_sb[blk][:, cp : cp + 2, 0, :],
                                x8r_sb[:, tcg, cp : cp + 2, :],
                                start=False,
                                stop=(cp == KC - 2),
                                perf_mode=DR,
                            )
                        if blk == "q":
                            nc.vector.tensor_scalar(
                                q8_sb[:, b, col : col + 512],
                                pst[:],
                                SW64,
                                bq_sb[:],
                                op0=MUL,
                                op1=ADD,
                            )
                        elif blk == "k":
                            k8 = k2_sb[:, 0, b, col : col + 512]
                            nc.vector.tensor_scalar(
                                k8, pst[:], SW64, bk_sb[:], op0=MUL, op1=ADD
                            )
                            # residual drops bk's own quantization residual
                            # (exact when bk == 0)
                            nc.vector.scalar_tensor_tensor(
                                k2_sb[:, 1, b, col : col + 512],
                                pst[:],
                                SW64,
                                k8,
                                op0=MUL,
                                op1=SUB,
                            )
                        else:
                            nc.vector.tensor_scalar(
                                vT_sb[:, b, col : col + 512],
                                pst[:],
                                SW64,
                                None,
                                op0=MUL,
                            )

                    return f

                def vfix():
                    bs = (tcg % 4) * 4
                    vtp = ps_qk.tile([P, 4, P], BF, name="vtp", tag="qk")
                    for j in range(4):
                        s = bs + j
                        nc.tensor.matmul(
                            vtp[:, j, :],
                            vT_sb[:, b, s * P : (s + 1) * P],
                            ident[:],
                            is_transpose=True,
                            start=(j == 0),
                            stop=(j == 3),
                        )
                    nc.vector.tensor_copy(
                        V2[:, b, bs : bs + 4, :].rearrange(
                            "p s (h x) -> p s h x", h=2
                        )[:, :, :, 1:65],
                        vtp[:].rearrange("p s (h d) -> p s h d", h=2),
                    )

                return [
                    chain_a("q", 0),
                    chain_a("q", 1),
                    chain_b("q"),
                    chain_a("k", 0),
                    chain_a("k", 1),
                    chain_b("k"),
                    chain_a("v", 0),
                    chain_a("v", 1),
                    chain_b("v"),
                    vfix,
                ]

            # ---- filler queue: (deadline_position, pe_cost_ns, closure) ----
            filler = []
            groups_done = [0]

            def pop_fillers(k):
                for _ in range(min(k, len(filler))):
                    filler.pop(0)[2]()

            def force_drain(upto):
                due = [u for u in filler if u[0] <= upto]
                filler[:] = [u for u in filler if u[0] > upto]
                for _, _, fn in due:
                    fn()

            def weave():
                # Pop fillers between score groups, bounded by a PE-time
                # budget (~1us) so the in-order PE never delays the next
                # group past ACT's ~1.8us exp backlog; deadline pressure
                # (urgency) can exceed the budget.
                groups_done[0] += 1
                if not filler:
                    return
                left = max(groups_total - groups_done[0], 1)
                total_cost = sum(f[1] for f in filler)
                budget = max(1000.0, total_cost / left)
                d0 = min(f[0] for f in filler)
                n_due = sum(1 for f in filler if f[0] <= d0)
                gb = (Gpre[d0] if d0 < len(Gpre) else groups_total) - groups_done[0]
                k_urgent = n_due if gb <= 0 else -(-n_due // gb)
                spent = 0.0
                popped = 0
                while filler and (spent < budget or popped < k_urgent):
                    _, c, fn = filler.pop(0)
                    fn()
                    spent += c
                    popped += 1

            # ---- scores for one 256-query superblock (fp8 DoubleRow) ----
            def emit_scores(b, sq):
                nk = 2 * sq + 2
                q0 = b * T + sq * SQ
                pt = {}
                for h in (0, 1):
                    pt[h] = ptp.tile([P, NQB, SQ], BF, name="ptt", tag="pt")
                for g in range(0, nk, 2):
                    diag = g == nk - 2
                    st = {}
                    for h in (0, 1):
                        st[h] = ps_st.tile([P, 512], F32, name="st", tag="st")
                    for j in (0, 1):
                        c = g + j
                        odd_diag = diag and j == 1
                        for h in (0, 1):
                            hp = h * DH
                            lhsT = k2_sb[hp : hp + DH, :, b, c * P : (c + 1) * P]
                            if odd_diag:
                                # valid queries (second 128) land at bank
                                # cols [256:384] so exp covers [0:384]
                                # contiguously; PV reads this chunk's qh=1
                                # data at pt[.., nk-1, 0:128]
                                q8b = (
                                    q8_sb[hp : hp + DH, b, sq * SQ + P : sq * SQ + SQ]
                                    .unsqueeze(1)
                                    .broadcast_to([DH, 2, P])
                                )
                                dst = st[h][:, SQ : SQ + P]
                            else:
                                q8b = (
                                    q8_sb[hp : hp + DH, b, sq * SQ : sq * SQ + SQ]
                                    .unsqueeze(1)
                                    .broadcast_to([DH, 2, SQ])
                                )
                                dst = st[h][:, j * SQ : (j + 1) * SQ]
                            nc.tensor.matmul(
                                dst,
                                lhsT,
                                q8b,
                                start=(j == 0),
                                stop=(j == 1) and not diag,
                                perf_mode=DR,
                            )
                    if diag:
                        # triangular -1536 mask: even chunk's first 128
                        # queries, odd chunk's (valid) last 128 queries
                        for h in (0, 1):
                            nc.tensor.matmul(
                                st[h][:, 0:P],
                                i8_sb[:],
                                m8_sb[:],
                                start=False,
                                stop=False,
                                perf_mode=DR,
                            )
                            nc.tensor.matmul(
                                st[h][:, SQ : SQ + P],
                                i8_sb[:],
                                m8_sb[:],
                                start=False,
                                stop=True,
                                perf_mode=DR,
                            )
                    ecols = SQ + P if diag else 2 * SQ
                    for h in (0, 1):
                        nc.scalar.activation(
                            pt[h][:, g : g + 2, :].rearrange("p a b -> p (a b)")[
                                :, 0:ecols
                            ],
                            st[h][:, 0:ecols],
                            mybir.ActivationFunctionType.Exp,
                            scale=0.125,
                        )
                    weave()
                return pt

            # ---- output work for one superblock: 4 filler units ----
            def make_output_units(
                b, sq, pt, pool=None, ptag=None, split_dma=False, use_act=False
            ):
                """use_act: offload normalize + half the proj evicts to the
                ACT engine — only valid after the last exp (tail blocks)."""
                nk = 2 * sq + 2
                held = {}
                pool = pool or ps_sm
                ptag = ptag or "sm"

                def pv_chain(hs, qh):
                    def f():
                        if hs == 0 and qh == 0:
                            held["pvt"] = pool.tile(
                                [P, 4 * 65], F32, name="pvt", tag=ptag
                            )
                        pvt = held["pvt"]
                        off = (2 * hs + qh) * 65
                        nck = nk - 1 if qh == 0 else nk
                        for c in range(nck):
                            # odd-diag chunk's valid (qh=1) data is
                            # stored at cols 0:128 (exp trim)
                            qoff = 0 if (qh == 1 and c == nk - 1) else qh * P
                            nc.tensor.matmul(
                                pvt[:, off : off + 65],
                                pt[hs][:, c, qoff : qoff + P],
                                V2[:, b, c, hs * 65 : hs * 65 + 65],
                                start=(hs == 0 and qh == 0 and c == 0),
                                stop=(c == nck - 1),
                            )

                    return f

                def norm():
                    pvt = held["pvt"]
                    osbs = []
                    for h in (0, 1):
                        for qh in (0, 1):
                            off = (2 * h + qh) * 65
                            r = rcp.tile([P, 1], F32, name="rr", tag="rr")
                            nc.vector.reciprocal(r[:], pvt[:, off : off + 1])
                            osb = osml.tile([P, DH], BF, name="osb")
                            if use_act:
                                nc.scalar.activation(
                                    osb[:],
                                    pvt[:, off + 1 : off + 65],
                                    mybir.ActivationFunctionType.Copy,
                                    scale=r[:],
                                )
                            else:
                                nc.vector.tensor_scalar_mul(
                                    osb[:], pvt[:, off + 1 : off + 65], r[:]
                                )
                            osbs.append((h, qh, osb))
                    held["osbs"] = osbs

                def tpev():
                    tp = pool.tile([P, SQ], BF, name="tp", tag=ptag)
                    for h, qh, osb in held["osbs"]:
                        hp = h * DH
                        nc.tensor.matmul(
                            tp[hp : hp + DH, qh * P : (qh + 1) * P],
                            osb[:],
                            ident[:],
                            is_transpose=True,
                            tile_position=(0, hp),
                            start=(qh == 0),
                            stop=(qh == 1),
                        )
                    nc.vector.tensor_copy(
                        attn_oT[:, b * T + sq * SQ : b * T + (sq + 1) * SQ], tp[:]
                    )

                def proj():
                    ys = ystage.tile([P, 2, D], BF, name="ys")
                    for half in range(2):
                        tt = b * (T // P) + 2 * sq + half
                        for nh in range(2):
                            psp = ps_pj.tile([P, 512], F32, name="psp", tag="pj")
                            nc.tensor.matmul(
                                psp[:],
                                attn_oT[:, tt * P : (tt + 1) * P],
                                wp_sb[:, nh * 512 : (nh + 1) * 512],
                                start=True,
                                stop=True,
                            )
                            if use_act and nh == 1:
                                nc.scalar.activation(
                                    ys[:, half, nh * 512 : (nh + 1) * 512],
                                    psp[:],
                                    mybir.ActivationFunctionType.Copy,
                                )
                            else:
                                nc.vector.tensor_copy(
                                    ys[:, half, nh * 512 : (nh + 1) * 512], psp[:]
                                )
                        if split_dma:
                            nc.sync.dma_start(
                                y_out[tt * P : (tt + 1) * P, :], ys[:, half, :]
                            )
                    if not split_dma:
                        t0 = (b * (T // P) + 2 * sq) * P
                        nc.sync.dma_start(
                            y_out[t0 : t0 + 2 * P, :].rearrange(
                                "(s p) d -> p s d", s=2
                            ),
                            ys[:],
                        )

                return [
                    pv_chain(0, 0),
                    pv_chain(0, 1),
                    pv_chain(1, 0),
                    pv_chain(1, 1),
                    norm,
                    tpev,
                    proj,
                ]

            # ---- main schedule ----
            # Emission order interleaves batch 1's small blocks into late
            # batch 0 so the ACT (exp) feed never thins out at the batch
            # boundary; positions are indices into this order.
            blocks = [
                (0, 0), (0, 1), (0, 2), (0, 3), (0, 4), (0, 5),
                (1, 0), (0, 6), (1, 1), (0, 7),
                (1, 2), (1, 3), (1, 7), (1, 6), (1, 5), (1, 4),
            ]
            Gpre = []
            acc = 0
            for _, ss in blocks:
                Gpre.append(acc)
                acc += ss + 1
            groups_total = acc
            # tc j's pieces enqueue after position p, force-drain before the
            # first position whose block reads tc j's q/k
            tc_after = {0: 1, 1: 2, 2: 3, 3: 4, 5: 5, 7: 6, 9: 7}
            tc_deadline = {1: 2, 2: 4, 3: 6, 4: 6, 5: 10, 6: 12, 7: 12}

            # tc0: q/k inline so B00's scores (the first exp feed) emit
            # ASAP; v work rides the filler queue (needed by outputs(B00)
            # at position 2)
            tc0 = make_tc_pieces(0)
            # q + k chains inline (B00 needs them), ordered to match DMA
            # arrival (x8s0, w_k, x8r s0) with warm matmuls plugging the
            # remaining DMA-latency holes so the PE clock never resets
            for piece, nw in zip(
                (tc0[0], tc0[1], tc0[3], tc0[4], tc0[2], tc0[5]),
                (0, 0, 8, 0, 8, 0),
            ):
                if nw:
                    warm(nw)
                piece()
            TC_COST = (430, 430, 430, 430, 430, 430, 430, 430, 430, 250)
            for piece in tc0[6:10]:  # v work rides the filler queue
                filler.append((2, 430, piece))
            pend = {}
            for i, (b, sq) in enumerate(blocks):
                force_drain(i)
                pt = emit_scores(b, sq)
                pend[i] = (b, sq, pt)
                if i - 2 in pend:
                    bb, ss, pp = pend.pop(i - 2)
                    pvc = (2 * ss + 2) * 27.0
                    costs = (pvc, pvc, pvc, pvc, 0.0, 270.0, 900.0)
                    for cst, u in zip(costs, make_output_units(bb, ss, pp)):
                        filler.append((i + 2, cst, u))
                if i in tc_after:
                    j = tc_after[i]
                    for cst, piece in zip(TC_COST, make_tc_pieces(j)):
                        filler.append((tc_deadline[j], cst, piece))
            # tail: interleave the last two blocks' stages so PE and DVE
            # overlap instead of serializing cross-engine round-trips. The
            # last block's PSUM tiles come from the (now idle) score banks
            # so the ps_sm single-slot ring doesn't force serialization,
            # and its y DMAs are split per 128-row tile.
            pop_fillers(len(filler))
            (b14, s14, p14), (b15, s15, p15) = pend[14], pend[15]
            u14 = make_output_units(b14, s14, p14, split_dma=True, use_act=True)
            u15 = make_output_units(
                b15, s15, p15, pool=ps_st, ptag="st", split_dma=True,
                use_act=True,
            )
            for fn in (u14[0], u14[1], u14[2], u14[3], u14[4], u15[0],
                       u15[1], u15[2], u15[3], u14[5], u14[6], u15[4],
                       u15[5], u15[6]):
                fn()

    nc.compile()
    return nc


def get_nc():
    global _CACHED_NC
    if _CACHED_NC is None:
        _CACHED_NC = build_nc()
    return _CACHED_NC


def make_in_maps(x, w_qkv, b_qkv, w_proj):
    bf = ml_dtypes.bfloat16
    e4 = ml_dtypes.float8_e4m3
    e5 = ml_dtypes.float8_e5m2
    x = np.asarray(x, dtype=np.float32).reshape(TA, D)
    w_qkv = np.asarray(w_qkv, dtype=np.float32)
    b_qkv = np.asarray(b_qkv, dtype=np.float32)
    w_proj = np.asarray(w_proj, dtype=np.float32)
    xT = np.ascontiguousarray(x.T)  # [D, TA] fp32, replicated

    def xlay(a):
        # device layout [P, NS, KC, 512]: chunk s contiguous per partition
        return np.ascontiguousarray(
            a.reshape(KC, P, TA // 512, 512).transpose(1, 2, 0, 3)
        )

    x8 = xT.astype(e4)
    x8r = xlay((xT - x8.astype(np.float32)).astype(e4))
    x8 = xlay(x8)

    def wpack(ws):
        # 64x scale keeps the 0.02-std weights out of e4m3's subnormal
        # range; the evict rescales by 1/64. Layout [P, KC, 2, P].
        ws = np.ascontiguousarray(ws) * 64.0
        w8 = ws.astype(e4)
        w8r = (ws - w8.astype(np.float32)).astype(e4)
        wp2 = np.stack([w8, w8r], axis=0)  # [2, D, P]
        return np.ascontiguousarray(
            wp2.reshape(2, KC, P, P).transpose(2, 1, 0, 3)
        )
    # fp8e5 identity + strict-upper-triangular additive mask (-1536); the
    # same [128,128] triangle serves both diagonal chunks. Slot 1 of each
    # DoubleRow pair is zeros.
    i8 = np.zeros((P, 2, P), dtype=e5)
    i8[:, 0, :] = np.eye(P, dtype=np.float32).astype(e5)
    m8 = np.zeros((P, 2, P), dtype=e5)
    kk = np.arange(P)[:, None]
    qq = np.arange(P)[None, :]
    m8[:, 0, :] = np.where(kk > qq, -1536.0, 0.0).astype(e5)
    in_maps = []
    for c in range(N_CORES):
        lo = 2 * c * DH  # first feature column of this core's 2 heads
        in_maps.append(
            {
                "x8": x8,
                "x8r": x8r,
                "wq": wpack(w_qkv[:, lo : lo + P]),
                "wk": wpack(w_qkv[:, D + lo : D + lo + P]),
                "wv": wpack(w_qkv[:, 2 * D + lo : 2 * D + lo + P]),
                "bq": np.ascontiguousarray(b_qkv[lo : lo + P][:, None]),
                "bk": np.ascontiguousarray(b_qkv[D + lo : D + lo + P][:, None]),
                "wp": np.ascontiguousarray(w_proj[lo : lo + P, :]).astype(bf),
                "i8": i8,
                "m8": m8,
            }
        )
    return in_maps


def gather(results, b_qkv, w_proj, b_proj):
    b_qkv = np.asarray(b_qkv, dtype=np.float32)
    w_proj = np.asarray(w_proj, dtype=np.float32)
    b_proj = np.asarray(b_proj, dtype=np.float32)
    y = np.zeros((TA, D), dtype=np.float32)
    for c in range(N_CORES):
        y += np.asarray(results[c]["y"], dtype=np.float32)
    # exact host-side fold of the v-bias and projection bias:
    # softmax rows sum to 1, so the v-bias passes through attention intact.
    y += b_qkv[2 * D : 3 * D] @ w_proj + b_proj
    return y.reshape(B, T, D)


def run(x, w_qkv, b_qkv, w_proj, b_proj, trace=False, **spmd_kwargs):
    nc = get_nc()
    in_maps = make_in_maps(x, w_qkv, b_qkv, w_proj)
    res = run_bass_kernel_spmd(
        nc, in_maps, list(range(N_CORES)), trace=trace, **spmd_kwargs
    )
    return gather(res.results, b_qkv, w_proj, b_proj), res


def kernel(x, w_qkv, b_qkv, w_proj, b_proj):
    y, _ = run(x, w_qkv, b_qkv, w_proj, b_proj)
    return y
